# revision 70
# baseline (speedup 1.0000x reference)
"""Distributed WeightedHGTConv kernel for 8 Trainium2 NeuronCores (Bass/Tile).

Strategy (node/dst sharding, self-contained):
  * Nodes are assigned to cores type-balanced: each node type's nodes are
    split evenly over the 8 cores and each per-core type slice is padded to
    a multiple of 128, so every 128-node tile holds a single type and the
    tile->type map is identical on all cores (one SPMD program).  Phase A
    is then ONE matmul per node tile (no per-type masking, 4x less xT
    traffic and PE work).  Every edge lives on the core that owns its dst
    node, so the segment softmax and scatter-add are core-local.
  * Host side: edges are sorted by dst and greedy-packed into 128-edge
    tiles such that no node's edge list spans a tile.  Per tile, a one-hot
    [edge, segment] matrix turns segment-sum of exp-scores (den) and of
    exp*V (num) into a single TensorEngine matmul.  Tiles are sorted by
    segment count so a per-mega segment cap (max over cores, baked into
    the program) shrinks the one-hot build and the stage write ~8x
    (segment rows only, not all 128 matmul rows).  All per-mega streams
    (idx/weights/segments) and the stage buffer are stored in mega-major
    [P, mega, ...] layout so every Phase-B DMA is one contiguous transfer.
  * Engine placement per the TRN2 cost model: the per-head score reduce
    is a packed-f16 pairwise-add tree (TensorTensor runs 2x on packed
    16-bit, TensorReduce has no fast mode); PSUM evacuations go to the
    Activation engine (PSUM operands force DVE to 1x).
  * Device phases: (A) per-tile Q/K/V projection (single matmul, type
    baked in), K|V written as one fp16 table; (AG) one AllGather
    replicates the K|V table; (B) per 128-edge tile: indirect-gather K|V
    rows by src; Q rows are gathered once per SEGMENT (floor(128/cap)
    tiles share one 128-row indirect DMA, amortizing the 994ns SWDGE
    fixed overhead ~7x) and expanded to edges by a per-tile TensorEngine
    matmul against a transposed one-hot built on-device (segment row
    replicated to all partitions by a K=1 ones-vector matmul, then
    is_equal vs a partition-index iota; the expand matmul contracts the
    full 128-row chunk at base partition 0 -- nonzero-base operand
    slices fault at runtime); dense per-edge relation/sign tables (fp16,
    host-precomputed), fused score reduce + exp, one-hot matmul, stage
    write in bf16; (D) softmax divide + skip-gate + layernorm over own
    nodes, written as f16.
  * Value-specialized fast paths (host-verified, with general fallbacks):
    zero Q/K/V biases skip the bias matmul, zero rel_bias skips the bias
    stream, identity layernorm params + uniform skip gate reduce Phase D
    to a scalar alpha and drop the gamma/beta stream.
  * Precision: fp16 storage for Q/K/V and relation tables (score path),
    bf16 stage (den|num), fp32 accumulation in PSUM and for the softmax
    division + layernorm; f16 output (host upcasts).
"""
import sys

sys.path.insert(0, "/opt/trn_rl_repo")

import numpy as np

CORES = 8
N_NODES = 50000
D = 128
H, DK = 8, 16
T, R = 4, 8
P = 128
MEGA = 16

_NC_CACHE = {}


# --------------------------------------------------------------------------
# host-side preparation
# --------------------------------------------------------------------------
def _host_prep(inputs, n, cores, mega):
    x = np.asarray(inputs["node_inp"], np.float32)
    nt = np.asarray(inputs["node_type"]).astype(np.int32)
    src = np.asarray(inputs["edge_index"][0]).astype(np.int64)
    dst = np.asarray(inputs["edge_index"][1]).astype(np.int64)
    et = np.asarray(inputs["edge_type"]).astype(np.int32)
    es = np.asarray(inputs["edge_sign"]).astype(np.int32)

    # --- type-balanced node->core assignment -------------------------------
    # Each type's nodes are split evenly over the cores and each per-core
    # type slice is padded to a multiple of P, so every 128-node tile has a
    # single node type AND the tile->type map is identical on all cores
    # (required: one SPMD program).
    type_splits = [np.array_split(np.nonzero(nt == t)[0], cores)
                   for t in range(T)]
    S_t = [((max(len(s[c]) for c in range(cores)) + P - 1) // P) * P
           for s in type_splits]
    np_nodes = int(sum(S_t))
    ntn = np_nodes // P
    tile_type = []
    for t in range(T):
        tile_type += [t] * (S_t[t] // P)
    tile_type = tuple(tile_type)

    l2g = np.full((cores, np_nodes), -1, np.int64)   # local row -> global id
    g2c = np.empty(n, np.int32)
    g2r = np.empty(n, np.int32)
    for c in range(cores):
        off = 0
        for t in range(T):
            ids = type_splits[t][c]
            l2g[c, off:off + len(ids)] = ids
            g2c[ids] = c
            g2r[ids] = off + np.arange(len(ids))
            off += S_t[t]

    sidx = np.where(es == -1, 0, np.where(es == 1, 1, 2)).astype(np.int32)
    cmb = (et * 3 + sidx).astype(np.int32)

    ones = np.ones((H, DK), np.float32)
    sk_all = np.stack([-ones, ones,
                       np.asarray(inputs["sign_k_neutral"], np.float32)], 0)
    sv_all = np.stack([-ones, ones,
                       np.asarray(inputs["sign_v_neutral"], np.float32)], 0)
    rel_q = np.asarray(inputs["rel_q"], np.float32)
    rel_k = np.asarray(inputs["rel_k"], np.float32)
    rel_v = np.asarray(inputs["rel_v"], np.float32)
    W2tab = (rel_q[:, None] * rel_k[:, None] * sk_all[None]).reshape(R * 3, D)
    Wvtab = (rel_v[:, None] * sv_all[None]).reshape(R * 3, D)
    bias4 = 4.0 * np.asarray(inputs["rel_bias"], np.float32)

    alpha = 1.0 / (1.0 + np.exp(-np.asarray(inputs["skip"], np.float32)))
    ln_g = np.asarray(inputs["ln_gamma"], np.float32)
    ln_b = np.asarray(inputs["ln_beta"], np.float32)
    has_bias = bool(
        np.any(np.asarray(inputs["bq"])) or np.any(np.asarray(inputs["bk"]))
        or np.any(np.asarray(inputs["bv"])))
    has_rbias = bool(np.any(np.asarray(inputs["rel_bias"])))
    simple_ln = bool(np.all(ln_g == 1.0) and np.all(ln_b == 0.0)
                     and np.all(alpha == alpha[0]))
    alpha0 = float(alpha[0])

    e_core = g2c[dst]
    e_row = g2r[dst].astype(np.int64)
    order = np.lexsort((e_row, e_core))
    dsts_r = e_row[order]
    e_core_s = e_core[order]
    srcs = src[order]
    cmbs = cmb[order]
    ets = et[order]

    core_lo = np.searchsorted(e_core_s, np.arange(cores))
    core_hi = np.searchsorted(e_core_s, np.arange(cores) + 1)

    per_core_tiles = []
    tile_counts = []
    for c in range(cores):
        lo, hi = core_lo[c], core_hi[c]
        d_loc = dsts_r[lo:hi]
        nodes, starts, counts = np.unique(d_loc, return_index=True,
                                          return_counts=True)
        tiles = []
        cur = []
        fill = 0
        for nid, st, ct in zip(nodes, starts, counts):
            assert ct <= P, f"node degree {ct} > {P}"
            if fill + ct > P:
                tiles.append(cur)
                cur = []
                fill = 0
            cur.append((int(nid), int(st), int(ct)))
            fill += ct
        if cur:
            tiles.append(cur)
        # fat tiles (many segments) first, so the per-mega segment cap
        # (max over the mega's tiles on all cores) stays tight
        tiles.sort(key=len, reverse=True)
        per_core_tiles.append(tiles)
        tile_counts.append(len(tiles))

    t_tiles = ((max(tile_counts) + 1 + mega - 1) // mega) * mega
    nmega = t_tiles // mega
    # per-mega segment cap, shared across cores (one SPMD program)
    caps = []
    for m in range(nmega):
        cap = 1
        for c in range(cores):
            for ti in range(m * mega, min((m + 1) * mega,
                                          len(per_core_tiles[c]))):
                cap = max(cap, len(per_core_tiles[c][ti]))
        caps.append(cap)
    caps = tuple(caps)
    offs = np.concatenate([[0], np.cumsum([mega * c for c in caps])])
    stage_rows = int(offs[-1])
    # q is gathered per SEGMENT (not per edge): each mega's 16*cap segment
    # q-rows are packed into 128-row chunks holding tpc = 128//cap whole
    # tiles each, so one indirect DMA serves tpc tiles
    # the expand matmul contracts the FULL 128-row chunk at base partition
    # 0, so slots pack densely: floor(128/cap) tiles per 128-row q chunk
    def _qgeom(cap):
        return max(1, P // cap), cap
    tpc = [_qgeom(c)[0] for c in caps]
    qstride = [_qgeom(c)[1] for c in caps]
    qchunks = [(mega + t - 1) // t for t in tpc]
    qoffs = np.concatenate([[0], np.cumsum(qchunks)])  # in 128-row chunks
    q_rows = int(qoffs[-1]) * P

    pc = []
    for c in range(cores):
        lo, hi = core_lo[c], core_hi[c]
        e_src = srcs[lo:hi]
        e_cmb = cmbs[lo:hi]
        e_et = ets[lo:hi]

        idx4 = np.zeros((t_tiles, P, 1), np.int32)
        ndmap = np.zeros((np_nodes, 1), np.int32)
        w2wv_e = np.zeros((t_tiles, P, 2 * D), np.float16)
        seg_e = np.full((t_tiles, P), 255, np.int16)
        seg2 = np.full((t_tiles, P), 255, np.float16)  # chunk-offset variant
        bias4_e = np.zeros((t_tiles, P, H), np.float32)
        qidx = np.zeros((q_rows, 1), np.int32)

        tiles = per_core_tiles[c]
        written = np.zeros(np_nodes, bool)
        for ti, tl in enumerate(tiles):
            m, g = ti // mega, ti % mega
            ro = (g % tpc[m]) * qstride[m]
            qbase = (int(qoffs[m]) + g // tpc[m]) * P + ro
            ep = 0
            for si, (nid, st, ct) in enumerate(tl):
                sl = slice(st, st + ct)
                rows = slice(ep, ep + ct)
                gsrc = e_src[sl]
                idx4[ti, rows, 0] = (g2c[gsrc] * np_nodes
                                     + g2r[gsrc]).astype(np.int32)
                w2wv_e[ti, rows, :D] = W2tab[e_cmb[sl]]
                w2wv_e[ti, rows, D:] = Wvtab[e_cmb[sl]]
                bias4_e[ti, rows] = bias4[e_et[sl]]
                seg_e[ti, rows] = si
                seg2[ti, rows] = ro + si
                qidx[qbase + si] = nid
                ndmap[nid] = int(offs[m]) + si * mega + g
                written[nid] = True
                ep += ct

        # isolated + pad local rows read a guaranteed-zero staging row of a
        # pad tile (its one-hot is all-zero for every live segment)
        zt = len(tiles)
        assert zt < t_tiles
        ndmap[~written] = int(offs[zt // mega]) + zt % mega

        # mega-major contiguous layouts so each mega's loads and the stage
        # write are single contiguous DMA transfers
        idx4 = np.ascontiguousarray(
            idx4.reshape(nmega, mega, P, 1).transpose(0, 2, 1, 3))
        w2wv_e = np.ascontiguousarray(
            w2wv_e.reshape(nmega, mega, P, 2 * D).transpose(0, 2, 1, 3))
        seg_e = np.ascontiguousarray(
            seg_e.reshape(nmega, mega, P).transpose(0, 2, 1))
        seg2 = np.ascontiguousarray(seg2.reshape(nmega, 1, mega * P))
        bias4_e = np.ascontiguousarray(
            bias4_e.reshape(nmega, mega, P, H).transpose(0, 2, 1, 3))

        own = l2g[c]
        valid = own >= 0
        x_own = np.zeros((np_nodes, D), np.float32)
        x_own[valid] = x[own[valid]]
        nt_own = np.zeros(np_nodes, np.int32)
        for i in range(ntn):
            nt_own[i * P:(i + 1) * P] = tile_type[i]
        xT = np.zeros((ntn, D, P), np.float16)
        for i in range(ntn):
            xT[i] = x_own[i * P:(i + 1) * P].T.astype(np.float16)

        a_n = alpha[nt_own].astype(np.float32)[:, None]
        a_n[~valid] = 1.0
        x1a = ((1.0 - a_n) * x_own).astype(np.float16)

        d = dict(idx4=idx4, w2wv_e=w2wv_e, seg_e=seg_e, seg2=seg2,
                 ndmap=ndmap, xT=xT, x1a=x1a, qidx=qidx)
        if has_rbias:
            d["bias4_e"] = bias4_e
        if not simple_ln:
            gbx = np.zeros((np_nodes, 2 * D), np.float16)
            gbx[:, :D] = ln_g[nt_own].astype(np.float16)
            gbx[:, D:] = ln_b[nt_own].astype(np.float16)
            gbx[~valid, :D] = 1.0
            gbx[~valid, D:] = 0.0
            d["alpha_n"] = a_n
            d["gbx"] = gbx
        pc.append(d)

    shared = dict(
        Wqkv=np.stack([np.concatenate(
            [np.asarray(inputs["Wq"], np.float32)[t],
             np.asarray(inputs["Wk"], np.float32)[t],
             np.asarray(inputs["Wv"], np.float32)[t]], axis=1)
            for t in range(T)]).astype(np.float16),
    )
    if has_bias:
        shared["bqkv"] = np.stack([np.concatenate(
            [np.asarray(inputs["bq"], np.float32)[t],
             np.asarray(inputs["bk"], np.float32)[t],
             np.asarray(inputs["bv"], np.float32)[t]])
            for t in range(T)]).astype(np.float16)
    meta = dict(t_tiles=t_tiles, mega=mega, cores=cores,
                np_nodes=np_nodes, ntn=ntn, tile_type=tile_type,
                has_bias=has_bias, has_rbias=has_rbias,
                simple_ln=simple_ln, alpha0=alpha0, l2g=l2g, caps=caps)
    return pc, shared, meta


# --------------------------------------------------------------------------
# device kernel
# --------------------------------------------------------------------------
def _build_nc(np_nodes, t_tiles, mega, cores, tile_type, has_bias,
              has_rbias, simple_ln, alpha0, caps, repeat=1):
    import concourse.bass as bass
    import concourse.tile as tile
    from concourse import mybir, bacc

    F16 = mybir.dt.float16
    BF16 = mybir.dt.bfloat16
    F32 = mybir.dt.float32
    I32 = mybir.dt.int32
    I16 = mybir.dt.int16

    ntn = np_nodes // P
    nmega = t_tiles // mega
    assert len(caps) == nmega
    offs = [0]
    for cp in caps:
        offs.append(offs[-1] + mega * cp)
    stage_rows = offs[-1]

    nc = bacc.Bacc()
    dp = nc.declare_dram_parameter

    xT = dp("xT", [ntn, D, P], F16, isOutput=False)
    Wqkv = dp("Wqkv", [T, D, 3 * D], F16, isOutput=False)
    if has_bias:
        bqkv = dp("bqkv", [T, 3 * D], F16, isOutput=False)
    def _qgeom(cap):
        return max(1, P // cap), cap
    tpc = [_qgeom(cp)[0] for cp in caps]
    qstride = [_qgeom(cp)[1] for cp in caps]
    qch = [(mega + t - 1) // t for t in tpc]
    qoffs = [0]
    for qc in qch:
        qoffs.append(qoffs[-1] + qc)
    q_rows = qoffs[-1] * P

    idx4 = dp("idx4", [nmega, P, mega, 1], I32, isOutput=False)
    w2wv_e = dp("w2wv_e", [nmega, P, mega, 2 * D], F16, isOutput=False)
    seg_e = dp("seg_e", [nmega, P, mega], I16, isOutput=False)
    seg2 = dp("seg2", [nmega, 1, mega * P], F16, isOutput=False)
    qidx = dp("qidx", [q_rows, 1], I32, isOutput=False)
    if has_rbias:
        bias4_e = dp("bias4_e", [nmega, P, mega, H], F32, isOutput=False)
    x1a = dp("x1a", [np_nodes, D], F16, isOutput=False)
    if not simple_ln:
        alpha_n = dp("alpha_n", [np_nodes, 1], F32, isOutput=False)
        gbx = dp("gbx", [np_nodes, 2 * D], F16, isOutput=False)
    ndmap = dp("ndmap", [np_nodes, 1], I32, isOutput=False)

    out = dp("out", [np_nodes, D], F16, isOutput=True)

    q_loc = nc.dram_tensor("q_loc", [np_nodes, D], F16)
    kv_own = nc.dram_tensor("kv_own", [np_nodes, 2 * D], F16)
    kv_all = nc.dram_tensor("kv_all", [cores * np_nodes, 2 * D], F16,
                            addr_space="Shared")
    stage = nc.dram_tensor("stage", [stage_rows, 8 + D], BF16)

    with tile.TileContext(nc) as tc:
        with tc.tile_pool(name="sb", bufs=2) as sb, \
             tc.tile_pool(name="sbc", bufs=1) as sbc:
          for _rep in range(repeat):

            # ---- Phase A: per-type projections of own nodes ----
            # (nodes are type-sorted + padded so each tile is single-type)
            wq_t = [sbc.tile([D, 3 * D], F16, tag=f"wq{t}", name=f"wq{t}")
                    for t in range(T)]
            for t in range(T):
                nc.sync.dma_start(out=wq_t[t][:], in_=Wqkv[t])
            if has_bias:
                bq_t = sbc.tile([T, 3 * D], F16, tag="bq")
                nc.sync.dma_start(out=bq_t[:], in_=bqkv[:])
                ones_r = sbc.tile([1, P], F16, tag="ones")
                nc.vector.memset(ones_r[:], 1.0)

            with tc.tile_pool(name="psA", bufs=2, space="PSUM") as psA:
                for i in range(ntn):
                    xm = sb.tile([D, P], F16, tag="xm")
                    nc.sync.dma_start(out=xm[:], in_=xT[i])
                    ps = psA.tile([P, 3 * D], F32, tag="psA")
                    nc.tensor.matmul(ps[:], lhsT=xm[:],
                                     rhs=wq_t[tile_type[i]][:],
                                     start=True, stop=not has_bias)
                    if has_bias:
                        nc.tensor.matmul(
                            ps[:], lhsT=ones_r[:],
                            rhs=bq_t[tile_type[i]:tile_type[i] + 1, :],
                            start=False, stop=True)
                    qo = sb.tile([P, D], F16, tag="qo")
                    nc.vector.tensor_copy(out=qo[:], in_=ps[:, 0:D])
                    kvo = sb.tile([P, 2 * D], F16, tag="kvo")
                    nc.vector.tensor_copy(out=kvo[:], in_=ps[:, D:3 * D])
                    nc.sync.dma_start(out=q_loc[i * P:(i + 1) * P], in_=qo[:])
                    nc.sync.dma_start(out=kv_own[i * P:(i + 1) * P],
                                      in_=kvo[:])

            # ---- replicate the K|V table ----
            nc.gpsimd.collective_compute(
                "AllGather", mybir.AluOpType.bypass,
                replica_groups=[list(range(cores))],
                ins=[kv_own[:]],
                outs=[kv_all[:]],
            )

            iv = sbc.tile([P, P], I16, tag="iv")
            nc.gpsimd.iota(iv[:], pattern=[[1, P]], base=0,
                           channel_multiplier=0)
            # every column = partition index (for the transposed one-hot)
            iv1 = sbc.tile([P, 1], I16, tag="iv1")
            nc.gpsimd.iota(iv1[:], pattern=[[1, 1]], base=0,
                           channel_multiplier=1)
            ivp = sbc.tile([P, mega * P], F32, tag="ivp")
            nc.vector.tensor_copy(
                out=ivp[:], in_=iv1[:, 0:1].to_broadcast([P, mega * P]))
            one1 = sbc.tile([1, P], F16, tag="one1")
            nc.vector.memset(one1[:], 1.0)

            # ---- Phase B: edge megatiles ----
            with tc.tile_pool(name="psB", bufs=1, space="PSUM") as psB, \
                 tc.tile_pool(name="psQ", bufs=1, space="PSUM") as psQ:
                for m in range(nmega):
                    cap = caps[m]
                    ix = sb.tile([P, mega, 1], I32, tag="ix", bufs=4)
                    nc.sync.dma_start(out=ix[:], in_=idx4[m])
                    ww = sb.tile([P, mega, 2 * D], F16, tag="ww", bufs=4)
                    nc.sync.dma_start(out=ww[:], in_=w2wv_e[m])
                    segt = sb.tile([P, mega], I16, tag="segt", bufs=4)
                    nc.sync.dma_start(out=segt[:], in_=seg_e[m])
                    oh = sb.tile([P, mega, P], BF16, tag="oh")
                    nc.vector.tensor_tensor(
                        out=oh[:, :, 0:cap],
                        in0=iv[:, None, 0:cap].to_broadcast([P, mega, cap]),
                        in1=segt[:, :, None].to_broadcast([P, mega, cap]),
                        op=mybir.AluOpType.is_equal)
                    # transposed one-hot [slot, edge] for the q expansion:
                    # replicate the chunk-offset segment row to all
                    # partitions with a K=1 ones-vector matmul (PE has
                    # headroom; partition_broadcast costs ~2.8us on Pool),
                    # then compare against the partition-index iota
                    # (integral values: f16/f32 equality is exact)
                    s2r = sb.tile([1, mega * P], F16, tag="s2r", bufs=3)
                    nc.sync.dma_start(out=s2r[:], in_=seg2[m])
                    s2p = psQ.tile([P, mega * P], F32, tag="psQ")
                    for j in range(4):
                        nc.tensor.matmul(
                            s2p[:, j * 512:(j + 1) * 512],
                            lhsT=one1[:],
                            rhs=s2r[0:1, j * 512:(j + 1) * 512],
                            start=True, stop=True)
                    oht = sb.tile([P, mega * P], F16, tag="oht", bufs=2)
                    nc.vector.tensor_tensor(
                        out=oht[:], in0=ivp[:], in1=s2p[:],
                        op=mybir.AluOpType.is_equal)
                    if has_rbias:
                        b4 = sb.tile([P, mega, H], F32, tag="b4", bufs=4)
                        nc.sync.dma_start(out=b4[:], in_=bias4_e[m])

                    kvg = sb.tile([P, mega, 2 * D], F16, tag="kvg")
                    for g in range(mega):
                        nc.gpsimd.indirect_dma_start(
                            out=kvg[:, g], out_offset=None,
                            in_=kv_all[:],
                            in_offset=bass.IndirectOffsetOnAxis(
                                ap=ix[:, g, 0:1], axis=0))
                    # q rows per segment: one gather serves tpc[m] tiles
                    qsg = sb.tile([P, qch[m], D], F16, tag="qsg", bufs=3)
                    qix = sb.tile([P, qch[m]], I32, tag="qix", bufs=3)
                    nc.sync.dma_start(
                        out=qix[:],
                        in_=qidx[qoffs[m] * P:qoffs[m + 1] * P]
                        .rearrange("(q p) c -> p (q c)", p=P))
                    for ch in range(qch[m]):
                        nc.gpsimd.indirect_dma_start(
                            out=qsg[:, ch], out_offset=None,
                            in_=q_loc[:],
                            in_offset=bass.IndirectOffsetOnAxis(
                                ap=qix[:, ch:ch + 1], axis=0))

                    kv2 = sb.tile([P, mega, 2 * D], F16, tag="kv2")
                    nc.vector.tensor_tensor(out=kv2[:], in0=kvg[:], in1=ww[:],
                                            op=mybir.AluOpType.mult)
                    # expand q from segments to edges (one matmul per tile):
                    # qex[e, d] = sum_s oht[s, e] * qseg[s, d]
                    sprod = sb.tile([P, mega, D], F16, tag="sprod")
                    hm = mega // 2
                    for hh in range(2):
                        # contract over the FULL 128-row q chunk at base
                        # partition 0 (nonzero-base matmul slices fault at
                        # runtime); rows outside the tile's slot range are
                        # zero in the one-hot, so the result is identical.
                        # 1KB PSUM stride per slot (proven alignment).
                        pq = psQ.tile([P, hm * 256], F32, tag="psQ")
                        for gl in range(hm):
                            g = hh * hm + gl
                            ch = g // tpc[m]
                            nc.tensor.matmul(
                                pq[:, gl * 256:gl * 256 + D],
                                lhsT=oht[0:P, g * P:(g + 1) * P],
                                rhs=qsg[:, ch],
                                start=True, stop=True)
                        nc.vector.tensor_tensor(
                            out=sprod[:, hh * hm:(hh + 1) * hm],
                            in0=kv2[:, hh * hm:(hh + 1) * hm, 0:D],
                            in1=pq[:].rearrange("p (m c) -> p m c",
                                                c=256)[:, :, 0:D],
                            op=mybir.AluOpType.mult)
                    # pairwise-add tree instead of reduce_sum: TensorTensor
                    # runs 2x on packed f16, TensorReduce has no fast mode
                    sp4 = sprod[:].rearrange("p m (h k) -> p m h k", k=DK)
                    tr8 = sb.tile([P, mega, H, 8], F16, tag="tr8")
                    nc.vector.tensor_tensor(
                        out=tr8[:], in0=sp4[:, :, :, 0:8],
                        in1=sp4[:, :, :, 8:16], op=mybir.AluOpType.add)
                    tr4 = sb.tile([P, mega, H, 4], F16, tag="tr4")
                    nc.vector.tensor_tensor(
                        out=tr4[:], in0=tr8[:, :, :, 0:4],
                        in1=tr8[:, :, :, 4:8], op=mybir.AluOpType.add)
                    tr2 = sb.tile([P, mega, H, 2], F16, tag="tr2")
                    nc.vector.tensor_tensor(
                        out=tr2[:], in0=tr4[:, :, :, 0:2],
                        in1=tr4[:, :, :, 2:4], op=mybir.AluOpType.add)
                    sred = sb.tile([P, mega, H], F32, tag="sred")
                    nc.vector.tensor_tensor(
                        out=sred[:, :, :, None],
                        in0=tr2[:, :, :, 0:1], in1=tr2[:, :, :, 1:2],
                        op=mybir.AluOpType.add)
                    if has_rbias:
                        s3 = sb.tile([P, mega, H], F32, tag="s3")
                        nc.vector.tensor_tensor(out=s3[:], in0=sred[:],
                                                in1=b4[:],
                                                op=mybir.AluOpType.add)
                    else:
                        s3 = sred
                    rt = sb.tile([P, mega, 8 + D], BF16, tag="rt")
                    nc.scalar.activation(
                        out=rt[:, :, 0:8], in_=s3[:],
                        func=mybir.ActivationFunctionType.Exp, scale=0.25)
                    nc.vector.tensor_tensor(
                        out=rt[:, :, 8:8 + D].rearrange(
                            "p m (h k) -> p m h k", k=DK),
                        in0=kv2[:, :, D:2 * D].rearrange(
                            "p m (h k) -> p m h k", k=DK),
                        in1=rt[:, :, 0:8, None].to_broadcast(
                            [P, mega, 8, DK]),
                        op=mybir.AluOpType.mult)

                    osc = sb.tile([P, mega, 8 + D], BF16, tag="osc")
                    for hh in range(2):
                        ps = psB.tile([P, hm * 256], F32, tag="psB")
                        for gl in range(hm):
                            g = hh * hm + gl
                            nc.tensor.matmul(
                                ps[0:cap, gl * 256:gl * 256 + 136],
                                lhsT=oh[:, g, 0:cap], rhs=rt[:, g],
                                start=True, stop=True)
                        # PSUM-source copies run 1x on DVE; Act has headroom
                        nc.scalar.activation(
                            out=osc[0:cap, hh * hm:(hh + 1) * hm],
                            in_=ps[0:cap]
                            .rearrange("p (m c) -> p m c", c=256)[:, :, 0:136],
                            func=mybir.ActivationFunctionType.Identity,
                            bias=0.0)
                    nc.sync.dma_start(
                        out=stage[offs[m]:offs[m + 1]].rearrange(
                            "(p m) c -> p m c", m=mega),
                        in_=osc[0:cap])

            # ---- Phase D: softmax divide + skip gate + layernorm ----
            for i in range(ntn):
                rs = slice(i * P, (i + 1) * P)
                ndix = sb.tile([P, 1], I32, tag="ndix")
                nc.sync.dma_start(out=ndix[:], in_=ndmap[rs])
                ac = sb.tile([P, 8 + D], BF16, tag="ac")
                nc.gpsimd.indirect_dma_start(
                    out=ac[:], out_offset=None, in_=stage[:],
                    in_offset=bass.IndirectOffsetOnAxis(ap=ndix[:], axis=0))
                xa = sb.tile([P, D], F16, tag="xa")
                nc.sync.dma_start(out=xa[:], in_=x1a[rs])
                if not simple_ln:
                    gb = sb.tile([P, 2 * D], F16, tag="gb")
                    nc.sync.dma_start(out=gb[:], in_=gbx[rs])
                    al = sb.tile([P, 1], F32, tag="al")
                    nc.sync.dma_start(out=al[:], in_=alpha_n[rs])

                rec = sb.tile([P, H], F32, tag="rec")
                nc.vector.tensor_scalar_add(rec[:], ac[:, 0:8], 1e-16)
                rec2 = sb.tile([P, H], F32, tag="rec2")
                nc.vector.reciprocal(rec2[:], rec[:])
                rec3 = sb.tile([P, H], F32, tag="rec3")
                if simple_ln:
                    nc.vector.tensor_scalar_mul(rec3[:], rec2[:], alpha0)
                else:
                    nc.vector.tensor_scalar_mul(rec3[:], rec2[:], al[:, 0:1])
                o1 = sb.tile([P, D], F32, tag="o1")
                nc.vector.tensor_tensor(
                    out=o1[:].rearrange("p (h k) -> p h k", k=DK),
                    in0=ac[:, 8:8 + D].rearrange("p (h k) -> p h k", k=DK),
                    in1=rec3[:, :, None].to_broadcast([P, H, DK]),
                    op=mybir.AluOpType.mult)
                pre = sb.tile([P, D], F32, tag="pre")
                nc.vector.tensor_tensor(out=pre[:], in0=o1[:], in1=xa[:],
                                        op=mybir.AluOpType.add)
                ssum = sb.tile([P, 1], F32, tag="ssum")
                cpy = sb.tile([P, D], F32, tag="cpy")
                nc.scalar.activation(
                    out=cpy[:], in_=pre[:],
                    func=mybir.ActivationFunctionType.Identity,
                    bias=0.0, accum_out=ssum[:])
                nmu = sb.tile([P, 1], F32, tag="nmu")
                nc.vector.tensor_scalar_mul(nmu[:], ssum[:], -1.0 / D)
                sq = sb.tile([P, D], F32, tag="sq")
                vsum = sb.tile([P, 1], F32, tag="vsum")
                nc.scalar.activation(
                    out=sq[:], in_=pre[:],
                    func=mybir.ActivationFunctionType.Square,
                    bias=nmu[:, 0:1], accum_out=vsum[:])
                veps = sb.tile([P, 1], F32, tag="veps")
                nc.vector.tensor_scalar(out=veps[:], in0=vsum[:],
                                        scalar1=1.0 / D, scalar2=1e-5,
                                        op0=mybir.AluOpType.mult,
                                        op1=mybir.AluOpType.add)
                sd = sb.tile([P, 1], F32, tag="sd")
                nc.scalar.activation(out=sd[:], in_=veps[:],
                                     func=mybir.ActivationFunctionType.Sqrt)
                rstd = sb.tile([P, 1], F32, tag="rstd")
                nc.vector.reciprocal(rstd[:], sd[:])
                d2 = sb.tile([P, D], F32, tag="d2")
                nc.scalar.activation(
                    out=d2[:], in_=pre[:],
                    func=mybir.ActivationFunctionType.Identity,
                    bias=nmu[:, 0:1])
                if simple_ln:
                    of2 = sb.tile([P, D], F16, tag="of2")
                    nc.vector.tensor_scalar_mul(of2[:], d2[:], rstd[:, 0:1])
                else:
                    gbs = sb.tile([P, D], F32, tag="gbs")
                    nc.vector.tensor_scalar_mul(gbs[:], gb[:, 0:D],
                                                rstd[:, 0:1])
                    of1 = sb.tile([P, D], F32, tag="of1")
                    nc.vector.tensor_tensor(out=of1[:], in0=d2[:],
                                            in1=gbs[:],
                                            op=mybir.AluOpType.mult)
                    of2 = sb.tile([P, D], F16, tag="of2")
                    nc.vector.tensor_tensor(out=of2[:], in0=of1[:],
                                            in1=gb[:, D:2 * D],
                                            op=mybir.AluOpType.add)
                nc.sync.dma_start(out=out[rs], in_=of2[:])

    nc.compile()
    return nc


def _in_map_for_core(pcd, shared):
    m = dict(shared)
    m.update(pcd)
    return m


# --------------------------------------------------------------------------
# entry point
# --------------------------------------------------------------------------
def kernel(**inputs):
    import jax
    # The on-disk XLA compilation cache does not key on the embedded BIR
    # payload of the bass_exec custom call; a stale hit returns a NEFF for a
    # different kernel body.  Always compile fresh.
    try:
        jax.config.update("jax_enable_compilation_cache", False)
    except Exception:
        pass
    from concourse.bass_utils import run_bass_kernel_spmd

    pc, shared, meta = _host_prep(inputs, N_NODES, CORES, MEGA)
    key = (meta["np_nodes"], meta["t_tiles"], meta["mega"], CORES,
           meta["tile_type"], meta["has_bias"], meta["has_rbias"],
           meta["simple_ln"], meta["alpha0"], meta["caps"])
    if key not in _NC_CACHE:
        _NC_CACHE[key] = _build_nc(*key)
    nc = _NC_CACHE[key]

    in_maps = [_in_map_for_core(pc[c], shared) for c in range(CORES)]
    res = run_bass_kernel_spmd(nc, in_maps, list(range(CORES)))

    l2g = meta["l2g"]
    out = np.zeros((N_NODES, D), np.float32)
    for c in range(CORES):
        own = l2g[c]
        valid = own >= 0
        out[own[valid]] = res.results[c]["out"][valid]
    return out



# revision 71
# speedup vs baseline: 1.3424x; 1.3424x over previous
"""Distributed WeightedHGTConv kernel for 8 Trainium2 NeuronCores (Bass/Tile).

Strategy (node/dst sharding, self-contained):
  * Nodes are assigned to cores type-balanced: each node type's nodes are
    split evenly over the 8 cores and each per-core type slice is padded to
    a multiple of 128, so every 128-node tile holds a single type and the
    tile->type map is identical on all cores (one SPMD program).  Phase A
    is then ONE matmul per node tile (no per-type masking, 4x less xT
    traffic and PE work).  Every edge lives on the core that owns its dst
    node, so the segment softmax and scatter-add are core-local.
  * Host side: edges are sorted by dst and greedy-packed into 128-edge
    tiles such that no node's edge list spans a tile.  Per tile, a one-hot
    [edge, segment] matrix turns segment-sum of exp-scores (den) and of
    exp*V (num) into a single TensorEngine matmul.  Tiles are sorted by
    segment count so a per-mega segment cap (max over cores, baked into
    the program) shrinks the one-hot build and the stage write ~8x
    (segment rows only, not all 128 matmul rows).  All per-mega streams
    (idx/weights/segments) and the stage buffer are stored in mega-major
    [P, mega, ...] layout so every Phase-B DMA is one contiguous transfer.
  * Engine placement per the TRN2 cost model: the per-head score reduce
    is a packed-f16 pairwise-add tree (TensorTensor runs 2x on packed
    16-bit, TensorReduce has no fast mode); PSUM evacuations go to the
    Activation engine (PSUM operands force DVE to 1x).
  * Device phases: (A) per-tile Q/K/V projection (single matmul, type
    baked in), K|V written as one fp16 table; (AG) one AllGather
    replicates the K|V table; (B) per 128-edge tile: indirect-gather K|V
    rows by src; Q rows are gathered once per SEGMENT (floor(128/cap)
    tiles share one 128-row indirect DMA, amortizing the 994ns SWDGE
    fixed overhead ~7x) and expanded to edges by a per-tile TensorEngine
    matmul against a transposed one-hot built on-device (segment row
    replicated to all partitions by a K=1 ones-vector matmul, then
    is_equal vs a partition-index iota; the expand matmul contracts the
    full 128-row chunk at base partition 0 -- nonzero-base operand
    slices fault at runtime); dense per-edge relation/sign tables (fp16,
    host-precomputed), fused score reduce + exp, one-hot matmul, stage
    write in bf16; (D) softmax divide + skip-gate + layernorm over own
    nodes, written as f16.
  * Value-specialized fast paths (host-verified, with general fallbacks):
    zero Q/K/V biases skip the bias matmul, zero rel_bias skips the bias
    stream, identity layernorm params + uniform skip gate reduce Phase D
    to a scalar alpha and drop the gamma/beta stream.
  * Precision: fp16 storage for Q/K/V and relation tables (score path),
    bf16 stage (den|num), fp32 accumulation in PSUM and for the softmax
    division + layernorm; f16 output (host upcasts).
"""
import sys

sys.path.insert(0, "/opt/trn_rl_repo")

import numpy as np

CORES = 8
N_NODES = 50000
D = 128
H, DK = 8, 16
T, R = 4, 8
P = 128
MEGA = 16

_NC_CACHE = {}


# --------------------------------------------------------------------------
# host-side preparation
# --------------------------------------------------------------------------
def _host_prep(inputs, n, cores, mega):
    x = np.asarray(inputs["node_inp"], np.float32)
    nt = np.asarray(inputs["node_type"]).astype(np.int32)
    src = np.asarray(inputs["edge_index"][0]).astype(np.int64)
    dst = np.asarray(inputs["edge_index"][1]).astype(np.int64)
    et = np.asarray(inputs["edge_type"]).astype(np.int32)
    es = np.asarray(inputs["edge_sign"]).astype(np.int32)

    # --- type-balanced node->core assignment -------------------------------
    # Each type's nodes are split evenly over the cores and each per-core
    # type slice is padded to a multiple of P, so every 128-node tile has a
    # single node type AND the tile->type map is identical on all cores
    # (required: one SPMD program).
    type_splits = [np.array_split(np.nonzero(nt == t)[0], cores)
                   for t in range(T)]
    S_t = [((max(len(s[c]) for c in range(cores)) + P - 1) // P) * P
           for s in type_splits]
    np_nodes = int(sum(S_t))
    ntn = np_nodes // P
    tile_type = []
    for t in range(T):
        tile_type += [t] * (S_t[t] // P)
    tile_type = tuple(tile_type)

    l2g = np.full((cores, np_nodes), -1, np.int64)   # local row -> global id
    g2c = np.empty(n, np.int32)
    g2r = np.empty(n, np.int32)
    for c in range(cores):
        off = 0
        for t in range(T):
            ids = type_splits[t][c]
            l2g[c, off:off + len(ids)] = ids
            g2c[ids] = c
            g2r[ids] = off + np.arange(len(ids))
            off += S_t[t]

    sidx = np.where(es == -1, 0, np.where(es == 1, 1, 2)).astype(np.int32)
    cmb = (et * 3 + sidx).astype(np.int32)

    ones = np.ones((H, DK), np.float32)
    sk_all = np.stack([-ones, ones,
                       np.asarray(inputs["sign_k_neutral"], np.float32)], 0)
    sv_all = np.stack([-ones, ones,
                       np.asarray(inputs["sign_v_neutral"], np.float32)], 0)
    rel_q = np.asarray(inputs["rel_q"], np.float32)
    rel_k = np.asarray(inputs["rel_k"], np.float32)
    rel_v = np.asarray(inputs["rel_v"], np.float32)
    W2tab = (rel_q[:, None] * rel_k[:, None] * sk_all[None]).reshape(R * 3, D)
    Wvtab = (rel_v[:, None] * sv_all[None]).reshape(R * 3, D)
    bias4 = 4.0 * np.asarray(inputs["rel_bias"], np.float32)

    alpha = 1.0 / (1.0 + np.exp(-np.asarray(inputs["skip"], np.float32)))
    ln_g = np.asarray(inputs["ln_gamma"], np.float32)
    ln_b = np.asarray(inputs["ln_beta"], np.float32)
    has_bias = bool(
        np.any(np.asarray(inputs["bq"])) or np.any(np.asarray(inputs["bk"]))
        or np.any(np.asarray(inputs["bv"])))
    has_rbias = bool(np.any(np.asarray(inputs["rel_bias"])))
    simple_ln = bool(np.all(ln_g == 1.0) and np.all(ln_b == 0.0)
                     and np.all(alpha == alpha[0]))
    alpha0 = float(alpha[0])

    e_core = g2c[dst]
    e_row = g2r[dst].astype(np.int64)
    order = np.lexsort((e_row, e_core))
    dsts_r = e_row[order]
    e_core_s = e_core[order]
    srcs = src[order]
    cmbs = cmb[order]
    ets = et[order]

    core_lo = np.searchsorted(e_core_s, np.arange(cores))
    core_hi = np.searchsorted(e_core_s, np.arange(cores) + 1)

    per_core_tiles = []
    tile_counts = []
    for c in range(cores):
        lo, hi = core_lo[c], core_hi[c]
        d_loc = dsts_r[lo:hi]
        nodes, starts, counts = np.unique(d_loc, return_index=True,
                                          return_counts=True)
        tiles = []
        cur = []
        fill = 0
        for nid, st, ct in zip(nodes, starts, counts):
            assert ct <= P, f"node degree {ct} > {P}"
            if fill + ct > P:
                tiles.append(cur)
                cur = []
                fill = 0
            cur.append((int(nid), int(st), int(ct)))
            fill += ct
        if cur:
            tiles.append(cur)
        # fat tiles (many segments) first, so the per-mega segment cap
        # (max over the mega's tiles on all cores) stays tight
        tiles.sort(key=len, reverse=True)
        per_core_tiles.append(tiles)
        tile_counts.append(len(tiles))

    t_tiles = ((max(tile_counts) + 1 + mega - 1) // mega) * mega
    nmega = t_tiles // mega
    # per-mega segment cap, shared across cores (one SPMD program)
    caps = []
    for m in range(nmega):
        cap = 1
        for c in range(cores):
            for ti in range(m * mega, min((m + 1) * mega,
                                          len(per_core_tiles[c]))):
                cap = max(cap, len(per_core_tiles[c][ti]))
        caps.append(cap)
    caps = tuple(caps)
    offs = np.concatenate([[0], np.cumsum([mega * c for c in caps])])
    stage_rows = int(offs[-1])
    # q is gathered per SEGMENT (not per edge): each mega's 16*cap segment
    # q-rows are packed into 128-row chunks holding tpc = 128//cap whole
    # tiles each, so one indirect DMA serves tpc tiles
    # the expand matmul contracts the FULL 128-row chunk at base partition
    # 0, so slots pack densely: floor(128/cap) tiles per 128-row q chunk
    def _qgeom(cap):
        return max(1, P // cap), cap
    tpc = [_qgeom(c)[0] for c in caps]
    qstride = [_qgeom(c)[1] for c in caps]
    qchunks = [(mega + t - 1) // t for t in tpc]
    qoffs = np.concatenate([[0], np.cumsum(qchunks)])  # in 128-row chunks
    q_rows = int(qoffs[-1]) * P

    pc = []
    for c in range(cores):
        lo, hi = core_lo[c], core_hi[c]
        e_src = srcs[lo:hi]
        e_cmb = cmbs[lo:hi]
        e_et = ets[lo:hi]

        idx4 = np.zeros((t_tiles, P, 1), np.int32)
        ndmap = np.zeros((np_nodes, 1), np.int32)
        w2wv_e = np.zeros((t_tiles, P, 2 * D), np.float16)
        seg_e = np.full((t_tiles, P), 255, np.int16)
        seg2 = np.full((t_tiles, P), 255, np.float16)  # chunk-offset variant
        bias4_e = np.zeros((t_tiles, P, H), np.float32)
        qidx = np.zeros((q_rows, 1), np.int32)

        tiles = per_core_tiles[c]
        written = np.zeros(np_nodes, bool)
        for ti, tl in enumerate(tiles):
            m, g = ti // mega, ti % mega
            ro = (g % tpc[m]) * qstride[m]
            qbase = (int(qoffs[m]) + g // tpc[m]) * P + ro
            ep = 0
            for si, (nid, st, ct) in enumerate(tl):
                sl = slice(st, st + ct)
                rows = slice(ep, ep + ct)
                gsrc = e_src[sl]
                idx4[ti, rows, 0] = (g2c[gsrc] * np_nodes
                                     + g2r[gsrc]).astype(np.int32)
                w2wv_e[ti, rows, :D] = W2tab[e_cmb[sl]]
                w2wv_e[ti, rows, D:] = Wvtab[e_cmb[sl]]
                bias4_e[ti, rows] = bias4[e_et[sl]]
                seg_e[ti, rows] = si
                seg2[ti, rows] = ro + si
                qidx[qbase + si] = nid
                ndmap[nid] = int(offs[m]) + si * mega + g
                written[nid] = True
                ep += ct

        # isolated + pad local rows read a guaranteed-zero staging row of a
        # pad tile (its one-hot is all-zero for every live segment)
        zt = len(tiles)
        assert zt < t_tiles
        ndmap[~written] = int(offs[zt // mega]) + zt % mega

        # mega-major contiguous layouts so each mega's loads and the stage
        # write are single contiguous DMA transfers
        idx4 = np.ascontiguousarray(
            idx4.reshape(nmega, mega, P, 1).transpose(0, 2, 1, 3))
        w2wv_e = np.ascontiguousarray(
            w2wv_e.reshape(nmega, mega, P, 2 * D).transpose(0, 2, 1, 3))
        seg_e = np.ascontiguousarray(
            seg_e.reshape(nmega, mega, P).transpose(0, 2, 1))
        seg2 = np.ascontiguousarray(seg2.reshape(nmega, 1, mega * P))
        bias4_e = np.ascontiguousarray(
            bias4_e.reshape(nmega, mega, P, H).transpose(0, 2, 1, 3))

        own = l2g[c]
        valid = own >= 0
        x_own = np.zeros((np_nodes, D), np.float32)
        x_own[valid] = x[own[valid]]
        nt_own = np.zeros(np_nodes, np.int32)
        for i in range(ntn):
            nt_own[i * P:(i + 1) * P] = tile_type[i]
        xT = np.zeros((ntn, D, P), np.float16)
        for i in range(ntn):
            xT[i] = x_own[i * P:(i + 1) * P].T.astype(np.float16)

        a_n = alpha[nt_own].astype(np.float32)[:, None]
        a_n[~valid] = 1.0
        x1a = ((1.0 - a_n) * x_own).astype(np.float16)

        d = dict(idx4=idx4, w2wv_e=w2wv_e, seg_e=seg_e, seg2=seg2,
                 ndmap=ndmap, xT=xT, x1a=x1a, qidx=qidx)
        if has_rbias:
            d["bias4_e"] = bias4_e
        if not simple_ln:
            gbx = np.zeros((np_nodes, 2 * D), np.float16)
            gbx[:, :D] = ln_g[nt_own].astype(np.float16)
            gbx[:, D:] = ln_b[nt_own].astype(np.float16)
            gbx[~valid, :D] = 1.0
            gbx[~valid, D:] = 0.0
            d["alpha_n"] = a_n
            d["gbx"] = gbx
        pc.append(d)

    shared = dict(
        Wqkv=np.stack([np.concatenate(
            [np.asarray(inputs["Wq"], np.float32)[t],
             np.asarray(inputs["Wk"], np.float32)[t],
             np.asarray(inputs["Wv"], np.float32)[t]], axis=1)
            for t in range(T)]).astype(np.float16),
    )
    if has_bias:
        shared["bqkv"] = np.stack([np.concatenate(
            [np.asarray(inputs["bq"], np.float32)[t],
             np.asarray(inputs["bk"], np.float32)[t],
             np.asarray(inputs["bv"], np.float32)[t]])
            for t in range(T)]).astype(np.float16)
    meta = dict(t_tiles=t_tiles, mega=mega, cores=cores,
                np_nodes=np_nodes, ntn=ntn, tile_type=tile_type,
                has_bias=has_bias, has_rbias=has_rbias,
                simple_ln=simple_ln, alpha0=alpha0, l2g=l2g, caps=caps)
    return pc, shared, meta


# --------------------------------------------------------------------------
# device kernel
# --------------------------------------------------------------------------
def _build_nc(np_nodes, t_tiles, mega, cores, tile_type, has_bias,
              has_rbias, simple_ln, alpha0, caps, repeat=1):
    import concourse.bass as bass
    import concourse.tile as tile
    from concourse import mybir, bacc

    F16 = mybir.dt.float16
    BF16 = mybir.dt.bfloat16
    F32 = mybir.dt.float32
    I32 = mybir.dt.int32
    I16 = mybir.dt.int16

    ntn = np_nodes // P
    nmega = t_tiles // mega
    assert len(caps) == nmega
    offs = [0]
    for cp in caps:
        offs.append(offs[-1] + mega * cp)
    stage_rows = offs[-1]

    nc = bacc.Bacc()
    dp = nc.declare_dram_parameter

    xT = dp("xT", [ntn, D, P], F16, isOutput=False)
    Wqkv = dp("Wqkv", [T, D, 3 * D], F16, isOutput=False)
    if has_bias:
        bqkv = dp("bqkv", [T, 3 * D], F16, isOutput=False)
    def _qgeom(cap):
        return max(1, P // cap), cap
    tpc = [_qgeom(cp)[0] for cp in caps]
    qstride = [_qgeom(cp)[1] for cp in caps]
    qch = [(mega + t - 1) // t for t in tpc]
    qoffs = [0]
    for qc in qch:
        qoffs.append(qoffs[-1] + qc)
    q_rows = qoffs[-1] * P

    idx4 = dp("idx4", [nmega, P, mega, 1], I32, isOutput=False)
    w2wv_e = dp("w2wv_e", [nmega, P, mega, 2 * D], F16, isOutput=False)
    seg_e = dp("seg_e", [nmega, P, mega], I16, isOutput=False)
    seg2 = dp("seg2", [nmega, 1, mega * P], F16, isOutput=False)
    qidx = dp("qidx", [q_rows, 1], I32, isOutput=False)
    if has_rbias:
        bias4_e = dp("bias4_e", [nmega, P, mega, H], F32, isOutput=False)
    x1a = dp("x1a", [np_nodes, D], F16, isOutput=False)
    if not simple_ln:
        alpha_n = dp("alpha_n", [np_nodes, 1], F32, isOutput=False)
        gbx = dp("gbx", [np_nodes, 2 * D], F16, isOutput=False)
    ndmap = dp("ndmap", [np_nodes, 1], I32, isOutput=False)

    out = dp("out", [np_nodes, D], F16, isOutput=True)

    q_loc = nc.dram_tensor("q_loc", [np_nodes, D], F16)
    kv_own = nc.dram_tensor("kv_own", [np_nodes, 2 * D], F16)
    kv_all = nc.dram_tensor("kv_all", [cores * np_nodes, 2 * D], F16,
                            addr_space="Shared")
    stage = nc.dram_tensor("stage", [stage_rows, 8 + D], BF16)

    with tile.TileContext(nc) as tc:
        with tc.tile_pool(name="sb", bufs=2) as sb, \
             tc.tile_pool(name="sbc", bufs=1) as sbc:
          for _rep in range(repeat):

            # ---- Phase A: per-type projections of own nodes ----
            # (nodes are type-sorted + padded so each tile is single-type)
            wq_t = [sbc.tile([D, 3 * D], F16, tag=f"wq{t}", name=f"wq{t}")
                    for t in range(T)]
            for t in range(T):
                nc.sync.dma_start(out=wq_t[t][:], in_=Wqkv[t])
            if has_bias:
                bq_t = sbc.tile([T, 3 * D], F16, tag="bq")
                nc.sync.dma_start(out=bq_t[:], in_=bqkv[:])
                ones_r = sbc.tile([1, P], F16, tag="ones")
                nc.vector.memset(ones_r[:], 1.0)

            with tc.tile_pool(name="psA", bufs=2, space="PSUM") as psA:
                for i in range(ntn):
                    xm = sb.tile([D, P], F16, tag="xm")
                    nc.sync.dma_start(out=xm[:], in_=xT[i])
                    ps = psA.tile([P, 3 * D], F32, tag="psA")
                    nc.tensor.matmul(ps[:], lhsT=xm[:],
                                     rhs=wq_t[tile_type[i]][:],
                                     start=True, stop=not has_bias)
                    if has_bias:
                        nc.tensor.matmul(
                            ps[:], lhsT=ones_r[:],
                            rhs=bq_t[tile_type[i]:tile_type[i] + 1, :],
                            start=False, stop=True)
                    qo = sb.tile([P, D], F16, tag="qo")
                    nc.vector.tensor_copy(out=qo[:], in_=ps[:, 0:D])
                    kvo = sb.tile([P, 2 * D], F16, tag="kvo")
                    nc.vector.tensor_copy(out=kvo[:], in_=ps[:, D:3 * D])
                    nc.sync.dma_start(out=q_loc[i * P:(i + 1) * P], in_=qo[:])
                    nc.sync.dma_start(out=kv_own[i * P:(i + 1) * P],
                                      in_=kvo[:])

            # ---- replicate the K|V table ----
            nc.gpsimd.collective_compute(
                "AllGather", mybir.AluOpType.bypass,
                replica_groups=[list(range(cores))],
                ins=[kv_own[:]],
                outs=[kv_all[:]],
            )

            iv = sbc.tile([P, P], I16, tag="iv")
            nc.gpsimd.iota(iv[:], pattern=[[1, P]], base=0,
                           channel_multiplier=0)
            # every column = partition index (for the transposed one-hot)
            iv1 = sbc.tile([P, 1], I16, tag="iv1")
            nc.gpsimd.iota(iv1[:], pattern=[[1, 1]], base=0,
                           channel_multiplier=1)
            ivp = sbc.tile([P, mega * P], F32, tag="ivp")
            nc.vector.tensor_copy(
                out=ivp[:], in_=iv1[:, 0:1].to_broadcast([P, mega * P]))
            one1 = sbc.tile([1, P], F16, tag="one1")
            nc.vector.memset(one1[:], 1.0)

            # ---- Phase B: edge megatiles ----
            # Hoist ALL q-segment gathers ahead of the kv gathers: they
            # read only the local q_loc, so the in-order Pool queue can
            # execute them while the AllGather is still in flight (the kv
            # gathers behind them wait on the collective's semaphore).
            nqc_total = qoffs[-1]
            qsg_all = sbc.tile([P, nqc_total, D], F16, tag="qsg_all")
            qix_all = sbc.tile([P, nqc_total], I32, tag="qix_all")
            nc.sync.dma_start(
                out=qix_all[:],
                in_=qidx[:].rearrange("(q p) c -> p (q c)", p=P))
            for qc_i in range(nqc_total):
                nc.gpsimd.indirect_dma_start(
                    out=qsg_all[:, qc_i], out_offset=None,
                    in_=q_loc[:],
                    in_offset=bass.IndirectOffsetOnAxis(
                        ap=qix_all[:, qc_i:qc_i + 1], axis=0))

            with tc.tile_pool(name="psB", bufs=1, space="PSUM") as psB, \
                 tc.tile_pool(name="psQ", bufs=1, space="PSUM") as psQ:
                for m in range(nmega):
                    cap = caps[m]
                    ix = sb.tile([P, mega, 1], I32, tag="ix", bufs=4)
                    nc.sync.dma_start(out=ix[:], in_=idx4[m])
                    ww = sb.tile([P, mega, 2 * D], F16, tag="ww", bufs=4)
                    nc.sync.dma_start(out=ww[:], in_=w2wv_e[m])
                    segt = sb.tile([P, mega], I16, tag="segt", bufs=4)
                    nc.sync.dma_start(out=segt[:], in_=seg_e[m])
                    oh = sb.tile([P, mega, P], BF16, tag="oh")
                    nc.vector.tensor_tensor(
                        out=oh[:, :, 0:cap],
                        in0=iv[:, None, 0:cap].to_broadcast([P, mega, cap]),
                        in1=segt[:, :, None].to_broadcast([P, mega, cap]),
                        op=mybir.AluOpType.is_equal)
                    # transposed one-hot [slot, edge] for the q expansion:
                    # replicate the chunk-offset segment row to all
                    # partitions with a K=1 ones-vector matmul (PE has
                    # headroom; partition_broadcast costs ~2.8us on Pool),
                    # then compare against the partition-index iota
                    # (integral values: f16/f32 equality is exact)
                    s2r = sb.tile([1, mega * P], F16, tag="s2r", bufs=3)
                    nc.sync.dma_start(out=s2r[:], in_=seg2[m])
                    s2p = psQ.tile([P, mega * P], F32, tag="psQ")
                    for j in range(4):
                        nc.tensor.matmul(
                            s2p[:, j * 512:(j + 1) * 512],
                            lhsT=one1[:],
                            rhs=s2r[0:1, j * 512:(j + 1) * 512],
                            start=True, stop=True)
                    oht = sb.tile([P, mega * P], F16, tag="oht", bufs=2)
                    nc.vector.tensor_tensor(
                        out=oht[:], in0=ivp[:], in1=s2p[:],
                        op=mybir.AluOpType.is_equal)
                    if has_rbias:
                        b4 = sb.tile([P, mega, H], F32, tag="b4", bufs=4)
                        nc.sync.dma_start(out=b4[:], in_=bias4_e[m])

                    kvg = sb.tile([P, mega, 2 * D], F16, tag="kvg")
                    for g in range(mega):
                        nc.gpsimd.indirect_dma_start(
                            out=kvg[:, g], out_offset=None,
                            in_=kv_all[:],
                            in_offset=bass.IndirectOffsetOnAxis(
                                ap=ix[:, g, 0:1], axis=0))
                    kv2 = sb.tile([P, mega, 2 * D], F16, tag="kv2")
                    nc.vector.tensor_tensor(out=kv2[:], in0=kvg[:], in1=ww[:],
                                            op=mybir.AluOpType.mult)
                    # expand q from segments to edges (one matmul per tile):
                    # qex[e, d] = sum_s oht[s, e] * qseg[s, d]
                    sprod = sb.tile([P, mega, D], F16, tag="sprod")
                    hm = mega // 2
                    for hh in range(2):
                        # contract over the FULL 128-row q chunk at base
                        # partition 0 (nonzero-base matmul slices fault at
                        # runtime); rows outside the tile's slot range are
                        # zero in the one-hot, so the result is identical.
                        # 1KB PSUM stride per slot (proven alignment).
                        pq = psQ.tile([P, hm * 256], F32, tag="psQ")
                        for gl in range(hm):
                            g = hh * hm + gl
                            ch = g // tpc[m]
                            nc.tensor.matmul(
                                pq[:, gl * 256:gl * 256 + D],
                                lhsT=oht[0:P, g * P:(g + 1) * P],
                                rhs=qsg_all[:, qoffs[m] + ch],
                                start=True, stop=True)
                        nc.vector.tensor_tensor(
                            out=sprod[:, hh * hm:(hh + 1) * hm],
                            in0=kv2[:, hh * hm:(hh + 1) * hm, 0:D],
                            in1=pq[:].rearrange("p (m c) -> p m c",
                                                c=256)[:, :, 0:D],
                            op=mybir.AluOpType.mult)
                    # pairwise-add tree instead of reduce_sum: TensorTensor
                    # runs 2x on packed f16, TensorReduce has no fast mode
                    sp4 = sprod[:].rearrange("p m (h k) -> p m h k", k=DK)
                    tr8 = sb.tile([P, mega, H, 8], F16, tag="tr8")
                    nc.vector.tensor_tensor(
                        out=tr8[:], in0=sp4[:, :, :, 0:8],
                        in1=sp4[:, :, :, 8:16], op=mybir.AluOpType.add)
                    tr4 = sb.tile([P, mega, H, 4], F16, tag="tr4")
                    nc.vector.tensor_tensor(
                        out=tr4[:], in0=tr8[:, :, :, 0:4],
                        in1=tr8[:, :, :, 4:8], op=mybir.AluOpType.add)
                    tr2 = sb.tile([P, mega, H, 2], F16, tag="tr2")
                    nc.vector.tensor_tensor(
                        out=tr2[:], in0=tr4[:, :, :, 0:2],
                        in1=tr4[:, :, :, 2:4], op=mybir.AluOpType.add)
                    sred = sb.tile([P, mega, H], F32, tag="sred")
                    nc.vector.tensor_tensor(
                        out=sred[:, :, :, None],
                        in0=tr2[:, :, :, 0:1], in1=tr2[:, :, :, 1:2],
                        op=mybir.AluOpType.add)
                    if has_rbias:
                        s3 = sb.tile([P, mega, H], F32, tag="s3")
                        nc.vector.tensor_tensor(out=s3[:], in0=sred[:],
                                                in1=b4[:],
                                                op=mybir.AluOpType.add)
                    else:
                        s3 = sred
                    rt = sb.tile([P, mega, 8 + D], BF16, tag="rt")
                    nc.scalar.activation(
                        out=rt[:, :, 0:8], in_=s3[:],
                        func=mybir.ActivationFunctionType.Exp, scale=0.25)
                    nc.vector.tensor_tensor(
                        out=rt[:, :, 8:8 + D].rearrange(
                            "p m (h k) -> p m h k", k=DK),
                        in0=kv2[:, :, D:2 * D].rearrange(
                            "p m (h k) -> p m h k", k=DK),
                        in1=rt[:, :, 0:8, None].to_broadcast(
                            [P, mega, 8, DK]),
                        op=mybir.AluOpType.mult)

                    osc = sb.tile([P, mega, 8 + D], BF16, tag="osc")
                    for hh in range(2):
                        ps = psB.tile([P, hm * 256], F32, tag="psB")
                        for gl in range(hm):
                            g = hh * hm + gl
                            nc.tensor.matmul(
                                ps[0:cap, gl * 256:gl * 256 + 136],
                                lhsT=oh[:, g, 0:cap], rhs=rt[:, g],
                                start=True, stop=True)
                        # PSUM-source copies run 1x on DVE; Act has headroom
                        nc.scalar.activation(
                            out=osc[0:cap, hh * hm:(hh + 1) * hm],
                            in_=ps[0:cap]
                            .rearrange("p (m c) -> p m c", c=256)[:, :, 0:136],
                            func=mybir.ActivationFunctionType.Identity,
                            bias=0.0)
                    nc.sync.dma_start(
                        out=stage[offs[m]:offs[m + 1]].rearrange(
                            "(p m) c -> p m c", m=mega),
                        in_=osc[0:cap])

            # ---- Phase D: softmax divide + skip gate + layernorm ----
            for i in range(ntn):
                rs = slice(i * P, (i + 1) * P)
                ndix = sb.tile([P, 1], I32, tag="ndix")
                nc.sync.dma_start(out=ndix[:], in_=ndmap[rs])
                ac = sb.tile([P, 8 + D], BF16, tag="ac")
                nc.gpsimd.indirect_dma_start(
                    out=ac[:], out_offset=None, in_=stage[:],
                    in_offset=bass.IndirectOffsetOnAxis(ap=ndix[:], axis=0))
                xa = sb.tile([P, D], F16, tag="xa")
                nc.sync.dma_start(out=xa[:], in_=x1a[rs])
                if not simple_ln:
                    gb = sb.tile([P, 2 * D], F16, tag="gb")
                    nc.sync.dma_start(out=gb[:], in_=gbx[rs])
                    al = sb.tile([P, 1], F32, tag="al")
                    nc.sync.dma_start(out=al[:], in_=alpha_n[rs])

                rec = sb.tile([P, H], F32, tag="rec")
                nc.vector.tensor_scalar_add(rec[:], ac[:, 0:8], 1e-16)
                rec2 = sb.tile([P, H], F32, tag="rec2")
                nc.vector.reciprocal(rec2[:], rec[:])
                rec3 = sb.tile([P, H], F32, tag="rec3")
                if simple_ln:
                    nc.vector.tensor_scalar_mul(rec3[:], rec2[:], alpha0)
                else:
                    nc.vector.tensor_scalar_mul(rec3[:], rec2[:], al[:, 0:1])
                o1 = sb.tile([P, D], F32, tag="o1")
                nc.vector.tensor_tensor(
                    out=o1[:].rearrange("p (h k) -> p h k", k=DK),
                    in0=ac[:, 8:8 + D].rearrange("p (h k) -> p h k", k=DK),
                    in1=rec3[:, :, None].to_broadcast([P, H, DK]),
                    op=mybir.AluOpType.mult)
                pre = sb.tile([P, D], F32, tag="pre")
                nc.vector.tensor_tensor(out=pre[:], in0=o1[:], in1=xa[:],
                                        op=mybir.AluOpType.add)
                ssum = sb.tile([P, 1], F32, tag="ssum")
                cpy = sb.tile([P, D], F32, tag="cpy")
                nc.scalar.activation(
                    out=cpy[:], in_=pre[:],
                    func=mybir.ActivationFunctionType.Identity,
                    bias=0.0, accum_out=ssum[:])
                nmu = sb.tile([P, 1], F32, tag="nmu")
                nc.vector.tensor_scalar_mul(nmu[:], ssum[:], -1.0 / D)
                sq = sb.tile([P, D], F32, tag="sq")
                vsum = sb.tile([P, 1], F32, tag="vsum")
                nc.scalar.activation(
                    out=sq[:], in_=pre[:],
                    func=mybir.ActivationFunctionType.Square,
                    bias=nmu[:, 0:1], accum_out=vsum[:])
                veps = sb.tile([P, 1], F32, tag="veps")
                nc.vector.tensor_scalar(out=veps[:], in0=vsum[:],
                                        scalar1=1.0 / D, scalar2=1e-5,
                                        op0=mybir.AluOpType.mult,
                                        op1=mybir.AluOpType.add)
                sd = sb.tile([P, 1], F32, tag="sd")
                nc.scalar.activation(out=sd[:], in_=veps[:],
                                     func=mybir.ActivationFunctionType.Sqrt)
                rstd = sb.tile([P, 1], F32, tag="rstd")
                nc.vector.reciprocal(rstd[:], sd[:])
                d2 = sb.tile([P, D], F32, tag="d2")
                nc.scalar.activation(
                    out=d2[:], in_=pre[:],
                    func=mybir.ActivationFunctionType.Identity,
                    bias=nmu[:, 0:1])
                if simple_ln:
                    of2 = sb.tile([P, D], F16, tag="of2")
                    nc.vector.tensor_scalar_mul(of2[:], d2[:], rstd[:, 0:1])
                else:
                    gbs = sb.tile([P, D], F32, tag="gbs")
                    nc.vector.tensor_scalar_mul(gbs[:], gb[:, 0:D],
                                                rstd[:, 0:1])
                    of1 = sb.tile([P, D], F32, tag="of1")
                    nc.vector.tensor_tensor(out=of1[:], in0=d2[:],
                                            in1=gbs[:],
                                            op=mybir.AluOpType.mult)
                    of2 = sb.tile([P, D], F16, tag="of2")
                    nc.vector.tensor_tensor(out=of2[:], in0=of1[:],
                                            in1=gb[:, D:2 * D],
                                            op=mybir.AluOpType.add)
                nc.sync.dma_start(out=out[rs], in_=of2[:])

    nc.compile()
    return nc


def _in_map_for_core(pcd, shared):
    m = dict(shared)
    m.update(pcd)
    return m


# --------------------------------------------------------------------------
# entry point
# --------------------------------------------------------------------------
def kernel(**inputs):
    import jax
    # The on-disk XLA compilation cache does not key on the embedded BIR
    # payload of the bass_exec custom call; a stale hit returns a NEFF for a
    # different kernel body.  Always compile fresh.
    try:
        jax.config.update("jax_enable_compilation_cache", False)
    except Exception:
        pass
    from concourse.bass_utils import run_bass_kernel_spmd

    pc, shared, meta = _host_prep(inputs, N_NODES, CORES, MEGA)
    key = (meta["np_nodes"], meta["t_tiles"], meta["mega"], CORES,
           meta["tile_type"], meta["has_bias"], meta["has_rbias"],
           meta["simple_ln"], meta["alpha0"], meta["caps"])
    if key not in _NC_CACHE:
        _NC_CACHE[key] = _build_nc(*key)
    nc = _NC_CACHE[key]

    in_maps = [_in_map_for_core(pc[c], shared) for c in range(CORES)]
    res = run_bass_kernel_spmd(nc, in_maps, list(range(CORES)))

    l2g = meta["l2g"]
    out = np.zeros((N_NODES, D), np.float32)
    for c in range(CORES):
        own = l2g[c]
        valid = own >= 0
        out[own[valid]] = res.results[c]["out"][valid]
    return out



# revision 72
# speedup vs baseline: 1.4083x; 1.0491x over previous
"""Distributed WeightedHGTConv kernel for 8 Trainium2 NeuronCores (Bass/Tile).

Strategy (node/dst sharding, self-contained):
  * Nodes are assigned to cores type-balanced: each node type's nodes are
    split evenly over the 8 cores and each per-core type slice is padded to
    a multiple of 128, so every 128-node tile holds a single type and the
    tile->type map is identical on all cores (one SPMD program).  Phase A
    is then ONE matmul per node tile (no per-type masking, 4x less xT
    traffic and PE work).  Every edge lives on the core that owns its dst
    node, so the segment softmax and scatter-add are core-local.
  * Host side: edges are sorted by dst and greedy-packed into 128-edge
    tiles such that no node's edge list spans a tile.  Per tile, a one-hot
    [edge, segment] matrix turns segment-sum of exp-scores (den) and of
    exp*V (num) into a single TensorEngine matmul.  Tiles are sorted by
    segment count so a per-mega segment cap (max over cores, baked into
    the program) shrinks the one-hot build and the stage write ~8x
    (segment rows only, not all 128 matmul rows).  All per-mega streams
    (idx/weights/segments) and the stage buffer are stored in mega-major
    [P, mega, ...] layout so every Phase-B DMA is one contiguous transfer.
  * Engine placement per the TRN2 cost model: the per-head score reduce
    is a packed-f16 pairwise-add tree (TensorTensor runs 2x on packed
    16-bit, TensorReduce has no fast mode); PSUM evacuations go to the
    Activation engine (PSUM operands force DVE to 1x).
  * Device phases: (A) per-tile Q/K/V projection (single matmul, type
    baked in), K|V written as one fp16 table; (AG) one AllGather
    replicates the K|V table; (B) per 128-edge tile: indirect-gather K|V
    rows by src; Q rows are gathered once per SEGMENT (floor(128/cap)
    tiles share one 128-row indirect DMA, amortizing the 994ns SWDGE
    fixed overhead ~7x) and expanded to edges by a per-tile TensorEngine
    matmul against a transposed one-hot built on-device (segment row
    replicated to all partitions by a K=1 ones-vector matmul, then
    is_equal vs a partition-index iota; the expand matmul contracts the
    full 128-row chunk at base partition 0 -- nonzero-base operand
    slices fault at runtime); dense per-edge relation/sign tables (fp16,
    host-precomputed), fused score reduce + exp, one-hot matmul, stage
    write in bf16; (D) softmax divide + skip-gate + layernorm over own
    nodes, written as f16.  All q-segment gathers are issued BEFORE the
    kv gathers so the in-order Pool queue executes them during the
    AllGather instead of head-of-line blocking behind the kv gathers'
    collective-wait.
  * Value-specialized fast paths (host-verified, with general fallbacks):
    zero Q/K/V biases skip the bias matmul, zero rel_bias skips the bias
    stream, identity layernorm params + uniform skip gate reduce Phase D
    to a scalar alpha and drop the gamma/beta stream.
  * Precision: fp16 storage for Q/K/V and relation tables (score path),
    bf16 stage (den|num), fp32 accumulation in PSUM and for the softmax
    division + layernorm; f16 output (host upcasts).
"""
import sys

sys.path.insert(0, "/opt/trn_rl_repo")

import numpy as np

CORES = 8
N_NODES = 50000
D = 128
H, DK = 8, 16
T, R = 4, 8
P = 128
MEGA = 16

_NC_CACHE = {}


# --------------------------------------------------------------------------
# host-side preparation
# --------------------------------------------------------------------------
def _host_prep(inputs, n, cores, mega):
    x = np.asarray(inputs["node_inp"], np.float32)
    nt = np.asarray(inputs["node_type"]).astype(np.int32)
    src = np.asarray(inputs["edge_index"][0]).astype(np.int64)
    dst = np.asarray(inputs["edge_index"][1]).astype(np.int64)
    et = np.asarray(inputs["edge_type"]).astype(np.int32)
    es = np.asarray(inputs["edge_sign"]).astype(np.int32)

    # --- type-balanced node->core assignment -------------------------------
    # Each type's nodes are split evenly over the cores and each per-core
    # type slice is padded to a multiple of P, so every 128-node tile has a
    # single node type AND the tile->type map is identical on all cores
    # (required: one SPMD program).
    type_splits = [np.array_split(np.nonzero(nt == t)[0], cores)
                   for t in range(T)]
    S_t = [((max(len(s[c]) for c in range(cores)) + P - 1) // P) * P
           for s in type_splits]
    np_nodes = int(sum(S_t))
    ntn = np_nodes // P
    tile_type = []
    for t in range(T):
        tile_type += [t] * (S_t[t] // P)
    tile_type = tuple(tile_type)

    l2g = np.full((cores, np_nodes), -1, np.int64)   # local row -> global id
    g2c = np.empty(n, np.int32)
    g2r = np.empty(n, np.int32)
    for c in range(cores):
        off = 0
        for t in range(T):
            ids = type_splits[t][c]
            l2g[c, off:off + len(ids)] = ids
            g2c[ids] = c
            g2r[ids] = off + np.arange(len(ids))
            off += S_t[t]

    sidx = np.where(es == -1, 0, np.where(es == 1, 1, 2)).astype(np.int32)
    cmb = (et * 3 + sidx).astype(np.int32)

    ones = np.ones((H, DK), np.float32)
    sk_all = np.stack([-ones, ones,
                       np.asarray(inputs["sign_k_neutral"], np.float32)], 0)
    sv_all = np.stack([-ones, ones,
                       np.asarray(inputs["sign_v_neutral"], np.float32)], 0)
    rel_q = np.asarray(inputs["rel_q"], np.float32)
    rel_k = np.asarray(inputs["rel_k"], np.float32)
    rel_v = np.asarray(inputs["rel_v"], np.float32)
    W2tab = (rel_q[:, None] * rel_k[:, None] * sk_all[None]).reshape(R * 3, D)
    Wvtab = (rel_v[:, None] * sv_all[None]).reshape(R * 3, D)
    bias4 = 4.0 * np.asarray(inputs["rel_bias"], np.float32)

    alpha = 1.0 / (1.0 + np.exp(-np.asarray(inputs["skip"], np.float32)))
    ln_g = np.asarray(inputs["ln_gamma"], np.float32)
    ln_b = np.asarray(inputs["ln_beta"], np.float32)
    has_bias = bool(
        np.any(np.asarray(inputs["bq"])) or np.any(np.asarray(inputs["bk"]))
        or np.any(np.asarray(inputs["bv"])))
    has_rbias = bool(np.any(np.asarray(inputs["rel_bias"])))
    simple_ln = bool(np.all(ln_g == 1.0) and np.all(ln_b == 0.0)
                     and np.all(alpha == alpha[0]))
    alpha0 = float(alpha[0])

    e_core = g2c[dst]
    e_row = g2r[dst].astype(np.int64)
    order = np.lexsort((e_row, e_core))
    dsts_r = e_row[order]
    e_core_s = e_core[order]
    srcs = src[order]
    cmbs = cmb[order]
    ets = et[order]

    core_lo = np.searchsorted(e_core_s, np.arange(cores))
    core_hi = np.searchsorted(e_core_s, np.arange(cores) + 1)

    per_core_tiles = []
    tile_counts = []
    for c in range(cores):
        lo, hi = core_lo[c], core_hi[c]
        d_loc = dsts_r[lo:hi]
        nodes, starts, counts = np.unique(d_loc, return_index=True,
                                          return_counts=True)
        tiles = []
        cur = []
        fill = 0
        for nid, st, ct in zip(nodes, starts, counts):
            assert ct <= P, f"node degree {ct} > {P}"
            if fill + ct > P:
                tiles.append(cur)
                cur = []
                fill = 0
            cur.append((int(nid), int(st), int(ct)))
            fill += ct
        if cur:
            tiles.append(cur)
        # fat tiles (many segments) first, so the per-mega segment cap
        # (max over the mega's tiles on all cores) stays tight
        tiles.sort(key=len, reverse=True)
        per_core_tiles.append(tiles)
        tile_counts.append(len(tiles))

    t_tiles = ((max(tile_counts) + 1 + mega - 1) // mega) * mega
    nmega = t_tiles // mega
    # per-mega segment cap, shared across cores (one SPMD program)
    caps = []
    for m in range(nmega):
        cap = 1
        for c in range(cores):
            for ti in range(m * mega, min((m + 1) * mega,
                                          len(per_core_tiles[c]))):
                cap = max(cap, len(per_core_tiles[c][ti]))
        caps.append(cap)
    caps = tuple(caps)
    offs = np.concatenate([[0], np.cumsum([mega * c for c in caps])])
    stage_rows = int(offs[-1])
    # q is gathered per SEGMENT (not per edge): each mega's 16*cap segment
    # q-rows are packed into 128-row chunks holding tpc = 128//cap whole
    # tiles each, so one indirect DMA serves tpc tiles
    # the expand matmul contracts the FULL 128-row chunk at base partition
    # 0, so slots pack densely: floor(128/cap) tiles per 128-row q chunk
    def _qgeom(cap):
        return max(1, P // cap), cap
    tpc = [_qgeom(c)[0] for c in caps]
    qstride = [_qgeom(c)[1] for c in caps]
    qchunks = [(mega + t - 1) // t for t in tpc]
    qoffs = np.concatenate([[0], np.cumsum(qchunks)])  # in 128-row chunks
    q_rows = int(qoffs[-1]) * P

    pc = []
    for c in range(cores):
        lo, hi = core_lo[c], core_hi[c]
        e_src = srcs[lo:hi]
        e_cmb = cmbs[lo:hi]
        e_et = ets[lo:hi]

        idx4 = np.zeros((t_tiles, P, 1), np.int32)
        ndmap = np.zeros((np_nodes, 1), np.int32)
        w2wv_e = np.zeros((t_tiles, P, 2 * D), np.float16)
        seg_e = np.full((t_tiles, P), 255, np.int16)
        seg2 = np.full((t_tiles, P), 255, np.float16)  # chunk-offset variant
        bias4_e = np.zeros((t_tiles, P, H), np.float32)
        qidx = np.zeros((q_rows, 1), np.int32)

        tiles = per_core_tiles[c]
        written = np.zeros(np_nodes, bool)
        for ti, tl in enumerate(tiles):
            m, g = ti // mega, ti % mega
            ro = (g % tpc[m]) * qstride[m]
            qbase = (int(qoffs[m]) + g // tpc[m]) * P + ro
            ep = 0
            for si, (nid, st, ct) in enumerate(tl):
                sl = slice(st, st + ct)
                rows = slice(ep, ep + ct)
                gsrc = e_src[sl]
                idx4[ti, rows, 0] = (g2c[gsrc] * np_nodes
                                     + g2r[gsrc]).astype(np.int32)
                w2wv_e[ti, rows, :D] = W2tab[e_cmb[sl]]
                w2wv_e[ti, rows, D:] = Wvtab[e_cmb[sl]]
                bias4_e[ti, rows] = bias4[e_et[sl]]
                seg_e[ti, rows] = si
                seg2[ti, rows] = ro + si
                qidx[qbase + si] = nid
                ndmap[nid] = int(offs[m]) + si * mega + g
                written[nid] = True
                ep += ct

        # isolated + pad local rows read a guaranteed-zero staging row of a
        # pad tile (its one-hot is all-zero for every live segment)
        zt = len(tiles)
        assert zt < t_tiles
        ndmap[~written] = int(offs[zt // mega]) + zt % mega

        # mega-major contiguous layouts so each mega's loads and the stage
        # write are single contiguous DMA transfers
        idx4 = np.ascontiguousarray(
            idx4.reshape(nmega, mega, P, 1).transpose(0, 2, 1, 3))
        w2wv_e = np.ascontiguousarray(
            w2wv_e.reshape(nmega, mega, P, 2 * D).transpose(0, 2, 1, 3))
        seg_e = np.ascontiguousarray(
            seg_e.reshape(nmega, mega, P).transpose(0, 2, 1))
        seg2 = np.ascontiguousarray(seg2.reshape(nmega, 1, mega * P))
        bias4_e = np.ascontiguousarray(
            bias4_e.reshape(nmega, mega, P, H).transpose(0, 2, 1, 3))

        own = l2g[c]
        valid = own >= 0
        x_own = np.zeros((np_nodes, D), np.float32)
        x_own[valid] = x[own[valid]]
        nt_own = np.zeros(np_nodes, np.int32)
        for i in range(ntn):
            nt_own[i * P:(i + 1) * P] = tile_type[i]
        xT = np.zeros((ntn, D, P), np.float16)
        for i in range(ntn):
            xT[i] = x_own[i * P:(i + 1) * P].T.astype(np.float16)

        a_n = alpha[nt_own].astype(np.float32)[:, None]
        a_n[~valid] = 1.0
        x1a = ((1.0 - a_n) * x_own).astype(np.float16)

        d = dict(idx4=idx4, w2wv_e=w2wv_e, seg_e=seg_e, seg2=seg2,
                 ndmap=ndmap, xT=xT, x1a=x1a, qidx=qidx)
        if has_rbias:
            d["bias4_e"] = bias4_e
        if not simple_ln:
            gbx = np.zeros((np_nodes, 2 * D), np.float16)
            gbx[:, :D] = ln_g[nt_own].astype(np.float16)
            gbx[:, D:] = ln_b[nt_own].astype(np.float16)
            gbx[~valid, :D] = 1.0
            gbx[~valid, D:] = 0.0
            d["alpha_n"] = a_n
            d["gbx"] = gbx
        pc.append(d)

    shared = dict(
        Wqkv=np.stack([np.concatenate(
            [np.asarray(inputs["Wq"], np.float32)[t],
             np.asarray(inputs["Wk"], np.float32)[t],
             np.asarray(inputs["Wv"], np.float32)[t]], axis=1)
            for t in range(T)]).astype(np.float16),
    )
    if has_bias:
        shared["bqkv"] = np.stack([np.concatenate(
            [np.asarray(inputs["bq"], np.float32)[t],
             np.asarray(inputs["bk"], np.float32)[t],
             np.asarray(inputs["bv"], np.float32)[t]])
            for t in range(T)]).astype(np.float16)
    meta = dict(t_tiles=t_tiles, mega=mega, cores=cores,
                np_nodes=np_nodes, ntn=ntn, tile_type=tile_type,
                has_bias=has_bias, has_rbias=has_rbias,
                simple_ln=simple_ln, alpha0=alpha0, l2g=l2g, caps=caps)
    return pc, shared, meta


# --------------------------------------------------------------------------
# device kernel
# --------------------------------------------------------------------------
def _build_nc(np_nodes, t_tiles, mega, cores, tile_type, has_bias,
              has_rbias, simple_ln, alpha0, caps, repeat=1):
    import concourse.bass as bass
    import concourse.tile as tile
    from concourse import mybir, bacc

    F16 = mybir.dt.float16
    BF16 = mybir.dt.bfloat16
    F32 = mybir.dt.float32
    I32 = mybir.dt.int32
    I16 = mybir.dt.int16

    ntn = np_nodes // P
    nmega = t_tiles // mega
    assert len(caps) == nmega
    offs = [0]
    for cp in caps:
        offs.append(offs[-1] + mega * cp)
    stage_rows = offs[-1]

    nc = bacc.Bacc()
    dp = nc.declare_dram_parameter

    xT = dp("xT", [ntn, D, P], F16, isOutput=False)
    Wqkv = dp("Wqkv", [T, D, 3 * D], F16, isOutput=False)
    if has_bias:
        bqkv = dp("bqkv", [T, 3 * D], F16, isOutput=False)
    def _qgeom(cap):
        return max(1, P // cap), cap
    tpc = [_qgeom(cp)[0] for cp in caps]
    qstride = [_qgeom(cp)[1] for cp in caps]
    qch = [(mega + t - 1) // t for t in tpc]
    qoffs = [0]
    for qc in qch:
        qoffs.append(qoffs[-1] + qc)
    q_rows = qoffs[-1] * P

    idx4 = dp("idx4", [nmega, P, mega, 1], I32, isOutput=False)
    w2wv_e = dp("w2wv_e", [nmega, P, mega, 2 * D], F16, isOutput=False)
    seg_e = dp("seg_e", [nmega, P, mega], I16, isOutput=False)
    seg2 = dp("seg2", [nmega, 1, mega * P], F16, isOutput=False)
    qidx = dp("qidx", [q_rows, 1], I32, isOutput=False)
    if has_rbias:
        bias4_e = dp("bias4_e", [nmega, P, mega, H], F32, isOutput=False)
    x1a = dp("x1a", [np_nodes, D], F16, isOutput=False)
    if not simple_ln:
        alpha_n = dp("alpha_n", [np_nodes, 1], F32, isOutput=False)
        gbx = dp("gbx", [np_nodes, 2 * D], F16, isOutput=False)
    ndmap = dp("ndmap", [np_nodes, 1], I32, isOutput=False)

    out = dp("out", [np_nodes, D], F16, isOutput=True)

    q_loc = nc.dram_tensor("q_loc", [np_nodes, D], F16)
    kv_own = nc.dram_tensor("kv_own", [np_nodes, 2 * D], F16)
    kv_all = nc.dram_tensor("kv_all", [cores * np_nodes, 2 * D], F16,
                            addr_space="Shared")
    stage = nc.dram_tensor("stage", [stage_rows, 8 + D], BF16)

    with tile.TileContext(nc) as tc:
        with tc.tile_pool(name="sb", bufs=2) as sb, \
             tc.tile_pool(name="sbc", bufs=1) as sbc:
          for _rep in range(repeat):

            # ---- Phase A: per-type projections of own nodes ----
            # (nodes are type-sorted + padded so each tile is single-type)
            wq_t = [sbc.tile([D, 3 * D], F16, tag=f"wq{t}", name=f"wq{t}")
                    for t in range(T)]
            for t in range(T):
                nc.sync.dma_start(out=wq_t[t][:], in_=Wqkv[t])
            if has_bias:
                bq_t = sbc.tile([T, 3 * D], F16, tag="bq")
                nc.sync.dma_start(out=bq_t[:], in_=bqkv[:])
                ones_r = sbc.tile([1, P], F16, tag="ones")
                nc.vector.memset(ones_r[:], 1.0)

            with tc.tile_pool(name="psA", bufs=2, space="PSUM") as psA:
                for i in range(ntn):
                    xm = sb.tile([D, P], F16, tag="xm")
                    nc.sync.dma_start(out=xm[:], in_=xT[i])
                    ps = psA.tile([P, 3 * D], F32, tag="psA")
                    nc.tensor.matmul(ps[:], lhsT=xm[:],
                                     rhs=wq_t[tile_type[i]][:],
                                     start=True, stop=not has_bias)
                    if has_bias:
                        nc.tensor.matmul(
                            ps[:], lhsT=ones_r[:],
                            rhs=bq_t[tile_type[i]:tile_type[i] + 1, :],
                            start=False, stop=True)
                    qo = sb.tile([P, D], F16, tag="qo")
                    nc.vector.tensor_copy(out=qo[:], in_=ps[:, 0:D])
                    kvo = sb.tile([P, 2 * D], F16, tag="kvo")
                    nc.vector.tensor_copy(out=kvo[:], in_=ps[:, D:3 * D])
                    nc.sync.dma_start(out=q_loc[i * P:(i + 1) * P], in_=qo[:])
                    nc.sync.dma_start(out=kv_own[i * P:(i + 1) * P],
                                      in_=kvo[:])

            # ---- replicate the K|V table ----
            nc.gpsimd.collective_compute(
                "AllGather", mybir.AluOpType.bypass,
                replica_groups=[list(range(cores))],
                ins=[kv_own[:]],
                outs=[kv_all[:]],
            )

            iv = sbc.tile([P, P], I16, tag="iv")
            nc.gpsimd.iota(iv[:], pattern=[[1, P]], base=0,
                           channel_multiplier=0)
            # every column = partition index (for the transposed one-hot)
            iv1 = sbc.tile([P, 1], I16, tag="iv1")
            nc.gpsimd.iota(iv1[:], pattern=[[1, 1]], base=0,
                           channel_multiplier=1)
            ivp = sbc.tile([P, mega * P], F32, tag="ivp")
            nc.vector.tensor_copy(
                out=ivp[:], in_=iv1[:, 0:1].to_broadcast([P, mega * P]))
            one1 = sbc.tile([1, P], F16, tag="one1")
            nc.vector.memset(one1[:], 1.0)

            # ---- Phase B: edge megatiles ----
            # Hoist ALL q-segment gathers ahead of the kv gathers: they
            # read only the local q_loc, so the in-order Pool queue can
            # execute them while the AllGather is still in flight (the kv
            # gathers behind them wait on the collective's semaphore).
            nqc_total = qoffs[-1]
            qsg_all = sbc.tile([P, nqc_total, D], F16, tag="qsg_all")
            qix_all = sbc.tile([P, nqc_total], I32, tag="qix_all")
            nc.sync.dma_start(
                out=qix_all[:],
                in_=qidx[:].rearrange("(q p) c -> p (q c)", p=P))
            for qc_i in range(nqc_total):
                nc.gpsimd.indirect_dma_start(
                    out=qsg_all[:, qc_i], out_offset=None,
                    in_=q_loc[:],
                    in_offset=bass.IndirectOffsetOnAxis(
                        ap=qix_all[:, qc_i:qc_i + 1], axis=0))

            with tc.tile_pool(name="psB", bufs=1, space="PSUM") as psB, \
                 tc.tile_pool(name="psQ", bufs=1, space="PSUM") as psQ:
                for m in range(nmega):
                    cap = caps[m]
                    ix = sb.tile([P, mega, 1], I32, tag="ix", bufs=4)
                    nc.sync.dma_start(out=ix[:], in_=idx4[m])
                    ww = sb.tile([P, mega, 2 * D], F16, tag="ww", bufs=4)
                    nc.sync.dma_start(out=ww[:], in_=w2wv_e[m])
                    segt = sb.tile([P, mega], I16, tag="segt", bufs=4)
                    nc.sync.dma_start(out=segt[:], in_=seg_e[m])
                    oh = sb.tile([P, mega, P], BF16, tag="oh")
                    nc.vector.tensor_tensor(
                        out=oh[:, :, 0:cap],
                        in0=iv[:, None, 0:cap].to_broadcast([P, mega, cap]),
                        in1=segt[:, :, None].to_broadcast([P, mega, cap]),
                        op=mybir.AluOpType.is_equal)
                    # transposed one-hot [slot, edge] for the q expansion:
                    # replicate the chunk-offset segment row to all
                    # partitions with a K=1 ones-vector matmul (PE has
                    # headroom; partition_broadcast costs ~2.8us on Pool),
                    # then compare against the partition-index iota
                    # (integral values: f16/f32 equality is exact)
                    s2r = sb.tile([1, mega * P], F16, tag="s2r", bufs=3)
                    nc.sync.dma_start(out=s2r[:], in_=seg2[m])
                    s2p = psQ.tile([P, mega * P], F32, tag="psQ")
                    for j in range(4):
                        nc.tensor.matmul(
                            s2p[:, j * 512:(j + 1) * 512],
                            lhsT=one1[:],
                            rhs=s2r[0:1, j * 512:(j + 1) * 512],
                            start=True, stop=True)
                    oht = sb.tile([P, mega * P], F16, tag="oht", bufs=2)
                    nc.vector.tensor_tensor(
                        out=oht[:], in0=ivp[:], in1=s2p[:],
                        op=mybir.AluOpType.is_equal)
                    if has_rbias:
                        b4 = sb.tile([P, mega, H], F32, tag="b4", bufs=4)
                        nc.sync.dma_start(out=b4[:], in_=bias4_e[m])

                    kvg = sb.tile([P, mega, 2 * D], F16, tag="kvg")
                    for g in range(mega):
                        nc.gpsimd.indirect_dma_start(
                            out=kvg[:, g], out_offset=None,
                            in_=kv_all[:],
                            in_offset=bass.IndirectOffsetOnAxis(
                                ap=ix[:, g, 0:1], axis=0))
                    kv2 = sb.tile([P, mega, 2 * D], F16, tag="kv2")
                    nc.vector.tensor_tensor(out=kv2[:], in0=kvg[:], in1=ww[:],
                                            op=mybir.AluOpType.mult)
                    # expand q from segments to edges (one matmul per tile):
                    # qex[e, d] = sum_s oht[s, e] * qseg[s, d]
                    sprod = sb.tile([P, mega, D], F16, tag="sprod")
                    hm = mega // 2
                    for hh in range(2):
                        # contract over the FULL 128-row q chunk at base
                        # partition 0 (nonzero-base matmul slices fault at
                        # runtime); rows outside the tile's slot range are
                        # zero in the one-hot, so the result is identical.
                        # 1KB PSUM stride per slot (proven alignment).
                        pq = psQ.tile([P, hm * 256], F32, tag="psQ")
                        for gl in range(hm):
                            g = hh * hm + gl
                            ch = g // tpc[m]
                            nc.tensor.matmul(
                                pq[:, gl * 256:gl * 256 + D],
                                lhsT=oht[0:P, g * P:(g + 1) * P],
                                rhs=qsg_all[:, qoffs[m] + ch],
                                start=True, stop=True)
                        nc.vector.tensor_tensor(
                            out=sprod[:, hh * hm:(hh + 1) * hm],
                            in0=kv2[:, hh * hm:(hh + 1) * hm, 0:D],
                            in1=pq[:].rearrange("p (m c) -> p m c",
                                                c=256)[:, :, 0:D],
                            op=mybir.AluOpType.mult)
                    # pairwise-add tree instead of reduce_sum: TensorTensor
                    # runs 2x on packed f16, TensorReduce has no fast mode
                    sp4 = sprod[:].rearrange("p m (h k) -> p m h k", k=DK)
                    tr8 = sb.tile([P, mega, H, 8], F16, tag="tr8")
                    nc.vector.tensor_tensor(
                        out=tr8[:], in0=sp4[:, :, :, 0:8],
                        in1=sp4[:, :, :, 8:16], op=mybir.AluOpType.add)
                    tr4 = sb.tile([P, mega, H, 4], F16, tag="tr4")
                    nc.vector.tensor_tensor(
                        out=tr4[:], in0=tr8[:, :, :, 0:4],
                        in1=tr8[:, :, :, 4:8], op=mybir.AluOpType.add)
                    tr2 = sb.tile([P, mega, H, 2], F16, tag="tr2")
                    nc.vector.tensor_tensor(
                        out=tr2[:], in0=tr4[:, :, :, 0:2],
                        in1=tr4[:, :, :, 2:4], op=mybir.AluOpType.add)
                    sred = sb.tile([P, mega, H], F32, tag="sred")
                    nc.vector.tensor_tensor(
                        out=sred[:, :, :, None],
                        in0=tr2[:, :, :, 0:1], in1=tr2[:, :, :, 1:2],
                        op=mybir.AluOpType.add)
                    if has_rbias:
                        s3 = sb.tile([P, mega, H], F32, tag="s3")
                        nc.vector.tensor_tensor(out=s3[:], in0=sred[:],
                                                in1=b4[:],
                                                op=mybir.AluOpType.add)
                    else:
                        s3 = sred
                    rt = sb.tile([P, mega, 8 + D], BF16, tag="rt")
                    nc.scalar.activation(
                        out=rt[:, :, 0:8], in_=s3[:],
                        func=mybir.ActivationFunctionType.Exp, scale=0.25)
                    nc.vector.tensor_tensor(
                        out=rt[:, :, 8:8 + D].rearrange(
                            "p m (h k) -> p m h k", k=DK),
                        in0=kv2[:, :, D:2 * D].rearrange(
                            "p m (h k) -> p m h k", k=DK),
                        in1=rt[:, :, 0:8, None].to_broadcast(
                            [P, mega, 8, DK]),
                        op=mybir.AluOpType.mult)

                    osc = sb.tile([P, mega, 8 + D], BF16, tag="osc")
                    for hh in range(2):
                        ps = psB.tile([P, hm * 256], F32, tag="psB")
                        for gl in range(hm):
                            g = hh * hm + gl
                            nc.tensor.matmul(
                                ps[0:cap, gl * 256:gl * 256 + 136],
                                lhsT=oh[:, g, 0:cap], rhs=rt[:, g],
                                start=True, stop=True)
                        # PSUM-source copies run 1x on DVE; Act has headroom
                        nc.scalar.activation(
                            out=osc[0:cap, hh * hm:(hh + 1) * hm],
                            in_=ps[0:cap]
                            .rearrange("p (m c) -> p m c", c=256)[:, :, 0:136],
                            func=mybir.ActivationFunctionType.Identity,
                            bias=0.0)
                    nc.sync.dma_start(
                        out=stage[offs[m]:offs[m + 1]].rearrange(
                            "(p m) c -> p m c", m=mega),
                        in_=osc[0:cap])

            # ---- Phase D: softmax divide + skip gate + layernorm ----
            for i in range(ntn):
                rs = slice(i * P, (i + 1) * P)
                ndix = sb.tile([P, 1], I32, tag="ndix")
                nc.sync.dma_start(out=ndix[:], in_=ndmap[rs])
                ac = sb.tile([P, 8 + D], BF16, tag="ac")
                nc.gpsimd.indirect_dma_start(
                    out=ac[:], out_offset=None, in_=stage[:],
                    in_offset=bass.IndirectOffsetOnAxis(ap=ndix[:], axis=0))
                xa = sb.tile([P, D], F16, tag="xa")
                nc.sync.dma_start(out=xa[:], in_=x1a[rs])
                if not simple_ln:
                    gb = sb.tile([P, 2 * D], F16, tag="gb")
                    nc.sync.dma_start(out=gb[:], in_=gbx[rs])
                    al = sb.tile([P, 1], F32, tag="al")
                    nc.sync.dma_start(out=al[:], in_=alpha_n[rs])

                rec = sb.tile([P, H], F32, tag="rec")
                nc.vector.tensor_scalar_add(rec[:], ac[:, 0:8], 1e-16)
                rec2 = sb.tile([P, H], F32, tag="rec2")
                nc.vector.reciprocal(rec2[:], rec[:])
                rec3 = sb.tile([P, H], F32, tag="rec3")
                if simple_ln:
                    nc.vector.tensor_scalar_mul(rec3[:], rec2[:], alpha0)
                else:
                    nc.vector.tensor_scalar_mul(rec3[:], rec2[:], al[:, 0:1])
                o1 = sb.tile([P, D], F32, tag="o1")
                nc.vector.tensor_tensor(
                    out=o1[:].rearrange("p (h k) -> p h k", k=DK),
                    in0=ac[:, 8:8 + D].rearrange("p (h k) -> p h k", k=DK),
                    in1=rec3[:, :, None].to_broadcast([P, H, DK]),
                    op=mybir.AluOpType.mult)
                pre = sb.tile([P, D], F32, tag="pre")
                nc.vector.tensor_tensor(out=pre[:], in0=o1[:], in1=xa[:],
                                        op=mybir.AluOpType.add)
                ssum = sb.tile([P, 1], F32, tag="ssum")
                cpy = sb.tile([P, D], F32, tag="cpy")
                nc.scalar.activation(
                    out=cpy[:], in_=pre[:],
                    func=mybir.ActivationFunctionType.Identity,
                    bias=0.0, accum_out=ssum[:])
                nmu = sb.tile([P, 1], F32, tag="nmu")
                nc.vector.tensor_scalar_mul(nmu[:], ssum[:], -1.0 / D)
                sq = sb.tile([P, D], F32, tag="sq")
                vsum = sb.tile([P, 1], F32, tag="vsum")
                nc.scalar.activation(
                    out=sq[:], in_=pre[:],
                    func=mybir.ActivationFunctionType.Square,
                    bias=nmu[:, 0:1], accum_out=vsum[:])
                veps = sb.tile([P, 1], F32, tag="veps")
                nc.vector.tensor_scalar(out=veps[:], in0=vsum[:],
                                        scalar1=1.0 / D, scalar2=1e-5,
                                        op0=mybir.AluOpType.mult,
                                        op1=mybir.AluOpType.add)
                sd = sb.tile([P, 1], F32, tag="sd")
                nc.scalar.activation(out=sd[:], in_=veps[:],
                                     func=mybir.ActivationFunctionType.Sqrt)
                rstd = sb.tile([P, 1], F32, tag="rstd")
                nc.vector.reciprocal(rstd[:], sd[:])
                d2 = sb.tile([P, D], F32, tag="d2")
                nc.scalar.activation(
                    out=d2[:], in_=pre[:],
                    func=mybir.ActivationFunctionType.Identity,
                    bias=nmu[:, 0:1])
                if simple_ln:
                    of2 = sb.tile([P, D], F16, tag="of2")
                    nc.vector.tensor_scalar_mul(of2[:], d2[:], rstd[:, 0:1])
                else:
                    gbs = sb.tile([P, D], F32, tag="gbs")
                    nc.vector.tensor_scalar_mul(gbs[:], gb[:, 0:D],
                                                rstd[:, 0:1])
                    of1 = sb.tile([P, D], F32, tag="of1")
                    nc.vector.tensor_tensor(out=of1[:], in0=d2[:],
                                            in1=gbs[:],
                                            op=mybir.AluOpType.mult)
                    of2 = sb.tile([P, D], F16, tag="of2")
                    nc.vector.tensor_tensor(out=of2[:], in0=of1[:],
                                            in1=gb[:, D:2 * D],
                                            op=mybir.AluOpType.add)
                nc.sync.dma_start(out=out[rs], in_=of2[:])

    nc.compile()
    return nc


def _in_map_for_core(pcd, shared):
    m = dict(shared)
    m.update(pcd)
    return m


# --------------------------------------------------------------------------
# entry point
# --------------------------------------------------------------------------
def kernel(**inputs):
    import jax
    # The on-disk XLA compilation cache does not key on the embedded BIR
    # payload of the bass_exec custom call; a stale hit returns a NEFF for a
    # different kernel body.  Always compile fresh.
    try:
        jax.config.update("jax_enable_compilation_cache", False)
    except Exception:
        pass
    from concourse.bass_utils import run_bass_kernel_spmd

    pc, shared, meta = _host_prep(inputs, N_NODES, CORES, MEGA)
    key = (meta["np_nodes"], meta["t_tiles"], meta["mega"], CORES,
           meta["tile_type"], meta["has_bias"], meta["has_rbias"],
           meta["simple_ln"], meta["alpha0"], meta["caps"])
    if key not in _NC_CACHE:
        _NC_CACHE[key] = _build_nc(*key)
    nc = _NC_CACHE[key]

    in_maps = [_in_map_for_core(pc[c], shared) for c in range(CORES)]
    res = run_bass_kernel_spmd(nc, in_maps, list(range(CORES)))

    l2g = meta["l2g"]
    out = np.zeros((N_NODES, D), np.float32)
    for c in range(CORES):
        own = l2g[c]
        valid = own >= 0
        out[own[valid]] = res.results[c]["out"][valid]
    return out

